# revision 1
# baseline (speedup 1.0000x reference)
"""Trainium2 Bass kernel for nn_BetweenClusterFC.

Computes out[e] = (emb_1[f[e]] @ W1 + b1) . (emb_2[t[e]] @ W2 + b2)
for E = 1.6M edges over N = 100k nodes, D_IN = 256, D_OUT = 128.

Strategy (8 NeuronCores, SPMD, full inputs in / full output out):
  - Nodes are split into 8 blocks of 12500.  Edges are assigned to cores by a
    (from-block-group, to-block-group) 4x2 rectangle: core c=(a,b) handles
    edges with from-node in blocks [4a..4a+3] and to-node in blocks
    [2b..2b+1].  Uniform (~200k edges/core), and each core only needs
    projections for 4 from-blocks + 2 to-blocks (75k nodes) instead of a
    fully replicated 200k -> far less HBM traffic.
  - Each core projects its 6 blocks (p = emb @ W + b) on the PE from
    host-pre-transposed embedding shards, writing p1/p2 tables to local DRAM.
  - Edges are bucketed host-side by (local from-block, local to-block) into
    8 buckets/core; per bucket both endpoint rows are fetched with the SWDGE
    dma_gather instruction (int16 local indices, 512B rows), then a DVE
    multiply + reduce produces the per-edge dot products.
  - The host applies the inverse edge permutation to assemble the output.

Written in raw Bass (explicit semaphores) — the Tile layer's generated sync
exceeds this toolchain's per-instruction wait-slot limits.
"""

import contextlib
import math

import numpy as np

import concourse.bass as bass
import concourse.mybir as mybir

# ---------------------------------------------------------------- constants
N_NODES = 100_000
D_IN = 256
D_OUT = 128
N_EDGES = 1_600_000
N_CORES = 8

NB = 12_500          # nodes per block
NBP = 12_544         # padded block rows (98 * 128)
NFB = 4              # from-blocks per core
NTB = 2              # to-blocks per core
NBUCKET = NFB * NTB  # 8 buckets per core

CAP = 26_624         # padded edge capacity per bucket (mean 25k, +10 sigma)
CALLS = [1024] * 26          # dma_gather call sizes (HW limit: <=1024 idxs/call)
assert sum(CALLS) == CAP
CALL_COLS = [g // 16 for g in CALLS]   # idx columns per call (wrapped by 16)
CALL_SLOTS = [g // 128 for g in CALLS]  # result slots per call
SLOT_TOT = CAP // 128                  # 208 result columns per bucket
COLS_PER_BUCKET = CAP // 16            # 1664 idx columns per bucket
IDX_COLS = NBUCKET * COLS_PER_BUCKET   # 13312

P1_ROWS = NFB * NBP  # 50176
P2_ROWS = NTB * NBP  # 25088

TILES1 = P1_ROWS // 128    # 392 node-tiles, table 1
TILES2 = P2_ROWS // 128    # 196 node-tiles, table 2
GROUPS1 = TILES1 // 4      # 98 psum groups
GROUPS2 = TILES2 // 4      # 49
NGROUP = GROUPS1 + GROUPS2  # 147
CHUNK_T = 14               # node-tiles per embT load chunk
NCH1 = TILES1 // CHUNK_T   # 28 chunks
NCH2 = TILES2 // CHUNK_T   # 14
NCHUNK = NCH1 + NCH2       # 42
EMB_COLS = CHUNK_T * 128   # 1792

NCALL = NBUCKET * len(CALLS)  # 56 gather calls per side

F32 = mybir.dt.float32
I16 = mybir.dt.int16
AX = mybir.AxisListType


# Processing order: p2 groups first, then p1 -> p-blocks finish progressively
# (p2b0@25, p2b1@49, p1b0@74, p1b1@98, p1b2@123, p1b3@147 positions), letting
# fi-major gather buckets start while later p1 blocks still project.
GSEQ = list(range(GROUPS1, NGROUP)) + list(range(GROUPS1))
CSEQ = list(range(NCH1, NCHUNK)) + list(range(NCH1))
CPOS = {cid: q for q, cid in enumerate(CSEQ)}
# pool gate positions: bucket group fi ready after this many processed groups
FI_READY = [49 + math.ceil(24.5 * (fi + 1)) for fi in range(NFB)]  # 74,98,123,147
INTERLEAVE_Q = 76  # start draining gather calls into the DVE stream here


def _chunk_of_tile(tg):
    """global tile index -> (global chunk id, table, local col0)."""
    if tg < TILES1:
        c = tg // CHUNK_T
        return c, 0, (tg % CHUNK_T) * 128
    t2 = tg - TILES1
    c = NCH1 + t2 // CHUNK_T
    return c, 1, (t2 % CHUNK_T) * 128


def _chunk_last_tile(c):
    """global chunk id -> global index of its last tile."""
    if c < NCH1:
        return (c + 1) * CHUNK_T - 1
    return TILES1 + (c - NCH1 + 1) * CHUNK_T - 1


def _chunk_src(c):
    """global chunk id -> (table, col0)."""
    if c < NCH1:
        return 0, c * EMB_COLS
    return 1, (c - NCH1) * EMB_COLS


# ---------------------------------------------------------------- device code
def build_bass(phase="all"):
    """phase: "all" | "proj" (p tables as outputs, no gather) |
    "gather" (p tables as inputs, no projection).  Non-"all" modes exist for
    hardware bring-up/debugging."""
    nc = bass.Bass()

    e1t = nc.dram_tensor("e1t", [D_IN, P1_ROWS], F32, kind="ExternalInput")
    e2t = nc.dram_tensor("e2t", [D_IN, P2_ROWS], F32, kind="ExternalInput")
    w1 = nc.dram_tensor("w1", [D_IN, D_OUT], F32, kind="ExternalInput")
    w2 = nc.dram_tensor("w2", [D_IN, D_OUT], F32, kind="ExternalInput")
    b1f = nc.dram_tensor("b1f", [128, 512], F32, kind="ExternalInput")
    b2f = nc.dram_tensor("b2f", [128, 512], F32, kind="ExternalInput")
    idxa = nc.dram_tensor("idxa", [128, IDX_COLS], I16, kind="ExternalInput")
    idxb = nc.dram_tensor("idxb", [128, IDX_COLS], I16, kind="ExternalInput")
    res = nc.dram_tensor("res", [NBUCKET, 128, SLOT_TOT], F32, kind="ExternalOutput")

    pkind = {"all": "Internal", "proj": "ExternalOutput", "gather": "ExternalInput"}[phase]
    p1d = nc.dram_tensor("p1d", [P1_ROWS, D_OUT], F32, kind=pkind)
    p2d = nc.dram_tensor("p2d", [P2_ROWS, D_OUT], F32, kind=pkind)
    pdst = (p1d, p2d)
    do_proj = phase in ("all", "proj")
    do_gather = phase in ("all", "gather")

    st = contextlib.ExitStack()
    with st:
        sb = lambda nm, shape, dt=F32: st.enter_context(nc.sbuf_tensor(nm, shape, dt))
        sem = lambda nm: st.enter_context(nc.semaphore(name=nm))

        w1c = sb("w1c", [128, 256])
        w2c = sb("w2c", [128, 256])
        bt = (sb("bt1", [128, 512]), sb("bt2", [128, 512]))
        idxt = (sb("idxta", [128, IDX_COLS], I16), sb("idxtb", [128, IDX_COLS], I16))
        et = [[sb(f"et_{p}_{k}", [128, EMB_COLS]) for k in range(2)]
              for p in range(2)]  # [parity][k]
        pv = [sb(f"pv{i}", [128, 512]) for i in range(4)]
        ps = [st.enter_context(nc.psum_tensor(f"ps{i}", [128, 512], F32))
              for i in range(4)]
        at = [sb(f"at{i}", [128, 8 * 128]) for i in range(4)]
        btg = [sb(f"btg{i}", [128, 8 * 128]) for i in range(4)]
        rt = [sb(f"rt{i}", [128, SLOT_TOT]) for i in range(4)]

        s_cl = sem("s_cl")               # const loads (8 dmas -> 128)
        s_load = (sem("s_load0"), sem("s_load1"))  # embT loads, by chunk parity
        s_mm = sem("s_mm")               # matmuls (+1 each; 2 per tile)
        s_bias = sem("s_bias")           # bias adds (+1 per group)
        s_pw = tuple(sem(f"s_pw{i}") for i in range(4))  # p-write dmas, by g%4
        s_g = tuple(sem(f"s_g{i}") for i in range(4))  # gathers, by k%4 (+16, 32/call)
        s_mul = sem("s_mul")             # muls (+1 per call)
        s_red = sem("s_red")             # reduces (+1 per call)
        s_out = tuple(sem(f"s_out{i}") for i in range(4))  # res dmas, by bk%4

        CONSTS = 8 * 16  # 8 const dmas

        block = st.enter_context(nc.Block())

        # ------------------------------------------------ SP: all HWDGE DMAs
        def _sp_proj(load_chunk, sync):
            load_chunk(0)
            load_chunk(1)
            next_cq = 2
            for q, g in enumerate(GSEQ):
                # look ahead: issue loads for chunks starting within 3 groups
                while next_cq < NCHUNK and next_cq * CHUNK_T <= (q + 3) * 4 + 3:
                    load_chunk(next_cq)
                    next_cq += 1
                sync.wait_ge(s_bias, q + 1)
                tab = 0 if g < GROUPS1 else 1
                r0 = g * 512 if tab == 0 else (g - GROUPS1) * 512
                sync.dma_start(
                    out=pdst[tab][r0:r0 + 512, :].rearrange("(t p) d -> p t d", p=128),
                    in_=pv[q % 4][:].rearrange("p (t d) -> p t d", d=128),
                ).then_inc(s_pw[q % 4], 16)
            if not do_gather:
                for r in range(4):
                    sync.wait_ge(s_pw[r], 16 * len(range(r, NGROUP, 4)))

        @block.sync
        def _(sync):
            for k in range(2):
                sync.dma_start(out=w1c[:, k * 128:(k + 1) * 128],
                               in_=w1[k * 128:(k + 1) * 128, :]).then_inc(s_cl, 16)
                sync.dma_start(out=w2c[:, k * 128:(k + 1) * 128],
                               in_=w2[k * 128:(k + 1) * 128, :]).then_inc(s_cl, 16)
            sync.dma_start(out=bt[0][:], in_=b1f[:]).then_inc(s_cl, 16)
            sync.dma_start(out=bt[1][:], in_=b2f[:]).then_inc(s_cl, 16)
            sync.dma_start(out=idxt[0][:], in_=idxa[:]).then_inc(s_cl, 16)
            sync.dma_start(out=idxt[1][:], in_=idxb[:]).then_inc(s_cl, 16)

            def load_chunk(cq):
                if cq >= 2:
                    # buffer cq%2 previously held chunk cq-2; wait until consumed
                    sync.wait_ge(s_mm, 2 * CHUNK_T * (cq - 1))
                tab, col0 = _chunk_src(CSEQ[cq])
                src = e1t if tab == 0 else e2t
                par = cq % 2
                sync.dma_start(out=et[par][0][:],
                               in_=src[0:128, col0:col0 + EMB_COLS]).then_inc(s_load[par], 16)
                sync.dma_start(out=et[par][1][:],
                               in_=src[128:256, col0:col0 + EMB_COLS]).then_inc(s_load[par], 16)

            if do_proj:
                _sp_proj(load_chunk, sync)

            if not do_gather:
                return
            for bk in range(NBUCKET):
                sync.wait_ge(s_red, len(CALLS) * (bk + 1))
                sync.dma_start(out=res[bk], in_=rt[bk % 4][:]).then_inc(s_out[bk % 4], 16)
            for r in range(4):
                sync.wait_ge(s_out[r], 16 * len(range(r, NBUCKET, 4)))

        # ------------------------------------------------ PE: projections
        @block.tensor
        def _(tensor):
            if not do_proj:
                return
            tensor.wait_ge(s_cl, CONSTS)
            for q, g in enumerate(GSEQ):
                tab = 0 if g < GROUPS1 else 1
                wc = w1c if tab == 0 else w2c
                for j in range(4):
                    tq = q * 4 + j
                    cid, _, col0 = _chunk_of_tile(g * 4 + j)
                    cq = CPOS[cid]
                    if tq == cq * CHUNK_T:  # first processed tile of chunk
                        tensor.wait_ge(s_load[cq % 2], 32 * (cq // 2 + 1))
                    if j == 0 and q >= 4:
                        tensor.wait_ge(s_bias, q - 3)  # psum bank q%4 free
                    out = ps[q % 4][:, j * 128:(j + 1) * 128]
                    tensor.matmul(out=out, lhsT=et[cq % 2][0][:, col0:col0 + 128],
                                  rhs=wc[:, 0:128], start=True, stop=False).then_inc(s_mm, 1)
                    tensor.matmul(out=out, lhsT=et[cq % 2][1][:, col0:col0 + 128],
                                  rhs=wc[:, 128:256], start=False, stop=True).then_inc(s_mm, 1)

        # ------------------------------------------------ DVE: bias + dot
        @block.vector
        def _(vector):
            def emit_call(k):
                bk, ci = k // len(CALLS), k % len(CALLS)
                S = CALL_SLOTS[ci]
                scol = sum(CALL_SLOTS[:ci])
                vector.wait_ge(s_g[k % 4], 32 * (k // 4 + 1))
                if ci == 0 and bk >= 4:
                    vector.wait_ge(s_out[bk % 4], 16 * (bk // 4))  # rt[bk%4] drained
                a3 = at[k % 4][:, :S * 128]
                b3 = btg[k % 4][:, :S * 128]
                vector.tensor_mul(out=a3, in0=a3, in1=b3).then_inc(s_mul, 1)
                vector.wait_ge(s_mul, k + 1)
                vector.reduce_sum(
                    out=rt[bk % 4][:, scol:scol + S],
                    in_=at[k % 4][:, :S * 128].rearrange("p (s d) -> p s d", d=128),
                    axis=AX.X,
                ).then_inc(s_red, 1)

            vector.wait_ge(s_cl, CONSTS)
            next_k = 0
            for q, g in enumerate(GSEQ) if do_proj else ():
                vector.wait_ge(s_mm, 8 * q + 8)
                if q >= 4:
                    vector.wait_ge(s_pw[q % 4], 16 * (q // 4))  # pv[q%4] drained
                tab = 0 if g < GROUPS1 else 1
                vector.tensor_add(out=pv[q % 4][:], in0=ps[q % 4][:],
                                  in1=bt[tab][:]).then_inc(s_bias, 1)
                if do_gather and q >= INTERLEAVE_Q and next_k < NCALL:
                    emit_call(next_k)
                    next_k += 1
            while do_gather and next_k < NCALL:
                emit_call(next_k)
                next_k += 1

        # ------------------------------------------------ Pool: gathers
        @block.gpsimd
        def _(gpsimd):
            if not do_gather:
                return
            from concourse import library_config
            gpsimd.load_library(library_config.mlp)
            regs = {gsz: gpsimd.to_reg(gsz) for gsz in sorted(set(CALLS))}
            gpsimd.wait_ge(s_cl, CONSTS)
            gated_fi = -1
            for k in range(NCALL):
                bk, ci = k // len(CALLS), k % len(CALLS)
                if do_proj and ci == 0 and bk // NTB > gated_fi:
                    gated_fi = bk // NTB
                    n = FI_READY[gated_fi]
                    for r in range(4):
                        gpsimd.wait_ge(s_pw[r], 16 * len(range(r, n, 4)))
                fi, ti = bk // NTB, bk % NTB
                gsz = CALLS[ci]
                S = CALL_SLOTS[ci]
                col0 = bk * COLS_PER_BUCKET + sum(CALL_COLS[:ci])
                ncols = CALL_COLS[ci]
                if k >= 4:
                    gpsimd.wait_ge(s_red, k - 3)  # at/bt[k%4] consumed
                gpsimd.dma_gather(
                    out_ap=at[k % 4][:, :S * 128].rearrange("p (s d) -> p s d", d=128),
                    in_ap=p1d[fi * NBP:(fi + 1) * NBP, :],
                    idxs_ap=idxt[0][:, col0:col0 + ncols],
                    num_idxs=gsz, num_idxs_reg=regs[gsz], elem_size=D_OUT,
                    queue_num=0,
                ).then_inc(s_g[k % 4], 16)
                gpsimd.dma_gather(
                    out_ap=btg[k % 4][:, :S * 128].rearrange("p (s d) -> p s d", d=128),
                    in_ap=p2d[ti * NBP:(ti + 1) * NBP, :],
                    idxs_ap=idxt[1][:, col0:col0 + ncols],
                    num_idxs=gsz, num_idxs_reg=regs[gsz], elem_size=D_OUT,
                    queue_num=0,
                ).then_inc(s_g[k % 4], 16)

    return nc


_NC_CACHE = None


def _get_nc():
    global _NC_CACHE
    if _NC_CACHE is None:
        nc = build_bass()
        from concourse.library_overlay import lower_extended_insts
        lower_extended_insts(nc)
        _NC_CACHE = nc
    return _NC_CACHE


# ---------------------------------------------------------------- host side
def _marshal(emb_1, emb_2, nodes_from_to, W1, b1, W2, b2):
    """Shard/bucket inputs per core.  Returns (in_maps, bookkeeping)."""
    f = np.asarray(nodes_from_to[:, 0], dtype=np.int64)
    t = np.asarray(nodes_from_to[:, 1], dtype=np.int64)
    emb_1 = np.ascontiguousarray(np.asarray(emb_1, dtype=np.float32))
    emb_2 = np.ascontiguousarray(np.asarray(emb_2, dtype=np.float32))
    W1 = np.asarray(W1, dtype=np.float32)
    W2 = np.asarray(W2, dtype=np.float32)
    b1 = np.asarray(b1, dtype=np.float32).reshape(-1)
    b2 = np.asarray(b2, dtype=np.float32).reshape(-1)

    core = (f // (NFB * NB)) * 4 + t // (NTB * NB)
    order0 = np.argsort(core, kind="stable")
    ccnt = np.bincount(core, minlength=N_CORES)
    coff = np.concatenate([[0], np.cumsum(ccnt)])

    b1f = np.tile(b1.reshape(1, D_OUT), (128, 4)).astype(np.float32)
    b2f = np.tile(b2.reshape(1, D_OUT), (128, 4)).astype(np.float32)

    in_maps, books = [], []
    for c in range(N_CORES):
        a, b = c // 4, c % 4
        sel = order0[coff[c]:coff[c + 1]]
        fc, tcv = f[sel], t[sel]
        fi = fc // NB - NFB * a
        ti = tcv // NB - NTB * b
        fl = (fc % NB).astype(np.int16)
        tl = (tcv % NB).astype(np.int16)
        bk = fi * NTB + ti
        o2 = np.argsort(bk, kind="stable")
        sel2, fl2, tl2 = sel[o2], fl[o2], tl[o2]
        cnts = np.bincount(bk, minlength=NBUCKET)
        if (cnts > CAP).any():
            raise RuntimeError(f"bucket overflow on core {c}: {cnts}")
        pos = np.concatenate([[0], np.cumsum(cnts)])

        slots_a = np.zeros((NBUCKET, CAP), np.int16)
        slots_b = np.zeros((NBUCKET, CAP), np.int16)
        for k in range(NBUCKET):
            slots_a[k, :cnts[k]] = fl2[pos[k]:pos[k + 1]]
            slots_b[k, :cnts[k]] = tl2[pos[k]:pos[k + 1]]
        # wrap by 16: idx i of a bucket at (partition i%16, col i//16),
        # replicated across the 8 groups of 16 partitions
        wa = slots_a.reshape(NBUCKET, CAP // 16, 16).transpose(0, 2, 1)
        wb = slots_b.reshape(NBUCKET, CAP // 16, 16).transpose(0, 2, 1)
        idxa = np.concatenate([np.tile(wa[k], (8, 1)) for k in range(NBUCKET)], axis=1)
        idxb = np.concatenate([np.tile(wb[k], (8, 1)) for k in range(NBUCKET)], axis=1)

        e1t = np.zeros((D_IN, P1_ROWS), np.float32)
        for i in range(NFB):
            blk = emb_1[(NFB * a + i) * NB:(NFB * a + i + 1) * NB]
            e1t[:, i * NBP:i * NBP + NB] = blk.T
        e2t = np.zeros((D_IN, P2_ROWS), np.float32)
        for i in range(NTB):
            blk = emb_2[(NTB * b + i) * NB:(NTB * b + i + 1) * NB]
            e2t[:, i * NBP:i * NBP + NB] = blk.T

        in_maps.append({
            "e1t": e1t, "e2t": e2t,
            "w1": W1, "w2": W2, "b1f": b1f, "b2f": b2f,
            "idxa": np.ascontiguousarray(idxa),
            "idxb": np.ascontiguousarray(idxb),
        })
        books.append((sel2, cnts, pos))
    return in_maps, books


def _unmarshal(results, books, n_edges):
    out = np.empty(n_edges, np.float32)
    scol0 = np.concatenate([[0], np.cumsum(CALL_SLOTS)])
    for c in range(N_CORES):
        sel2, cnts, pos = books[c]
        r = results[c]["res"]  # [NBUCKET, 128, SLOT_TOT]
        for k in range(NBUCKET):
            if cnts[k] == 0:
                continue
            arr = r[k]
            stream = np.concatenate([
                arr[:, scol0[ci]:scol0[ci] + CALL_SLOTS[ci]].T.reshape(-1)
                for ci in range(len(CALLS))
            ])
            out[sel2[pos[k]:pos[k + 1]]] = stream[:cnts[k]]
    return out


def _run(inputs, trace=False, **run_kwargs):
    from concourse.bass_utils import run_bass_kernel_spmd

    nc = _get_nc()
    in_maps, books = _marshal(**inputs)
    r = run_bass_kernel_spmd(
        nc, in_maps, core_ids=list(range(N_CORES)), trace=trace, **run_kwargs
    )
    out = _unmarshal(r.results, books, len(inputs["nodes_from_to"]))
    return out, r


def kernel(**inputs) -> np.ndarray:
    out, _ = _run(inputs, trace=False)
    return out



# revision 19
# speedup vs baseline: 2.3855x; 2.3855x over previous
"""Trainium2 Bass kernel for nn_BetweenClusterFC.

Computes out[e] = (emb_1[f[e]] @ W1 + b1) . (emb_2[t[e]] @ W2 + b2)
for E = 1.6M edges over N = 100k nodes, D_IN = 256, D_OUT = 128.

Strategy (8 NeuronCores, SPMD, full inputs in / full output out):
  - Edges are assigned to cores by a (from-half, to-quarter) 2x4 rectangle:
    core c=(a,b) handles edges with from-node in [50000a, 50000(a+1)) and
    to-node in [25000b, 25000(b+1)).  Each core projects its 75k nodes and
    stores p1/p2 fp16 row tables in local DRAM (tolerance is 2e-2; fp16
    keeps the end-to-end error ~7e-4).
  - The PE projects 128-row tiles (2 matmuls per tile, contraction 256 =
    2x128, optional per-tile bias matmul); the ACT engine drains PSUM to
    fp16 and issues the p-table writes, keeping the DVE free for dots.
  - The embedding shards are host-transposed AND column-permuted so each
    PSUM drain maps to one contiguous 1024-byte DRAM run per partition
    (rows 4p+j of each 512-row group live on partition p).
  - Edges are bucketed by (from-piece, to-piece), pieces being
    chunk-aligned slices of the node tables; the projection stream
    alternates to/from pieces (small pieces first) so the first buckets
    unlock ~20us in.  Endpoint rows are fetched with SWDGE dma_gather
    reading the tables as int32[64] rows (raw 256-byte row moves); dots
    are a DVE fp16 multiply + 4-level fold tree + reduce.
  - Bucket capacities are mean + MARGIN*sigma; if an input distribution
    overflows them the host falls back to a wider-margin build (slow
    recompile, still correct).
  - The host applies the inverse edge permutation to assemble the output.

Written in raw Bass (explicit semaphores).
"""

import contextlib
import math
from types import SimpleNamespace

import numpy as np

import concourse.bass as bass
import concourse.mybir as mybir

# ---------------------------------------------------------------- constants
N_NODES = 100_000
D_IN = 256
D_OUT = 128
N_EDGES = 1_600_000
N_CORES = 8

FSPAN = 50_000        # from-node span per core (2 groups)
TSPAN = 25_000        # to-node span per core (4 groups)
P1_ROWS = 50_176      # padded from-table rows (98 groups of 512)
P2_ROWS = 25_088      # padded to-table rows (49 groups)
G1 = P1_ROWS // 512   # 98
G2 = P2_ROWS // 512   # 49
NGROUP = G1 + G2      # 147

CHUNK_T = 14                  # tiles per embT load chunk
EMB_COLS = CHUNK_T * 128      # 1792
NCH1 = P1_ROWS // EMB_COLS    # 28 chunks, from table
NCH2 = P2_ROWS // EMB_COLS    # 14 chunks, to table

# Node pieces (chunk-aligned, small first).  8 x 4 pieces -> 32 buckets.
F_PIECE_CH = [2, 2, 4, 4, 4, 4, 4, 4]
T_PIECE_CH = [2, 2, 5, 5]
NF = len(F_PIECE_CH)
NT = len(T_PIECE_CH)
NBUCKET = NF * NT

F_OFF = (np.concatenate([[0], np.cumsum(F_PIECE_CH)]) * EMB_COLS).tolist()
T_OFF = (np.concatenate([[0], np.cumsum(T_PIECE_CH)]) * EMB_COLS).tolist()
F_GROUPS = [math.ceil(F_OFF[i + 1] / 512) for i in range(NF)]
T_GROUPS = [math.ceil(T_OFF[i + 1] / 512) for i in range(NT)]

MAX_CALL = 1024     # idxs per dma_gather call (HW limit: <=1024 idxs/call)
MAX_BATCH_E = 4608  # edges per DVE batch
EARLY_DRAINS = 40   # PSUM drains done by the (initially idle) DVE
IDX_DMA_PIECES = 6  # idx table upload pieces per side (interleaved w/ chunks)
MARGIN_TIERS = (3.5, 8.0)  # bucket-cap sigma margins (tier 1 = fallback)

F32 = mybir.dt.float32
F16 = mybir.dt.float16
I16 = mybir.dt.int16
I32 = mybir.dt.int32
AX = mybir.AxisListType

# ------------------------------------------------ projection stream order
PIECE_STREAM = []
for _k in range(max(NF, NT)):
    if _k < NT:
        PIECE_STREAM.append((1, _k))  # tab 1 = to/p2
    if _k < NF:
        PIECE_STREAM.append((0, _k))  # tab 0 = from/p1

GSEQ = []  # [(tab, group)]
_done = {0: 0, 1: 0}
for _tab, _pc in PIECE_STREAM:
    _end = F_GROUPS[_pc] if _tab == 0 else T_GROUPS[_pc]
    for _g in range(_done[_tab], _end):
        GSEQ.append((_tab, _g))
    _done[_tab] = _end
assert len(GSEQ) == NGROUP
GPOS = {tg: q for q, tg in enumerate(GSEQ)}

PIECE_POS = {}
for _tab, _npc, _pg in ((0, NF, F_GROUPS), (1, NT, T_GROUPS)):
    for _pc in range(_npc):
        PIECE_POS[(_tab, _pc)] = GPOS[(_tab, _pg[_pc] - 1)] + 1

BUCKET_POS = {}
for _i in range(NF):
    for _j in range(NT):
        BUCKET_POS[_i * NT + _j] = max(PIECE_POS[(0, _i)], PIECE_POS[(1, _j)])
BUCKET_ORDER = sorted(range(NBUCKET), key=lambda b: (BUCKET_POS[b], b))

# chunk order by first use in the stream
_first = {}
for _q, (_tab, _g) in enumerate(GSEQ):
    for _c in range((_g * 4) // CHUNK_T, (_g * 4 + 3) // CHUNK_T + 1):
        _first.setdefault((_tab, _c), _q)
CSEQ = sorted(_first, key=lambda tc: _first[tc])
assert len(CSEQ) == NCH1 + NCH2

CHUNK_LAST = {}
for _q, (_tab, _g) in enumerate(GSEQ):
    for _c in range((_g * 4) // CHUNK_T, (_g * 4 + 3) // CHUNK_T + 1):
        CHUNK_LAST[(_tab, _c)] = _q


def margin_caps(margin):
    """Formula capacities: mean + margin sigma (used when no counts known)."""
    mean_edges = N_EDGES / N_CORES
    caps = []
    for i in range(NF):
        fr = min(F_OFF[i + 1], FSPAN) - min(F_OFF[i], FSPAN)
        for j in range(NT):
            tr = min(T_OFF[j + 1], TSPAN) - min(T_OFF[j], TSPAN)
            mean = mean_edges * (fr / FSPAN) * (tr / TSPAN)
            cap = mean + margin * math.sqrt(mean) + 64
            caps.append(int(math.ceil(cap / 128) * 128))
    return caps


def make_sched(caps):
    """Call/batch/gather schedule for explicit bucket capacities."""
    caps = [max(128, c) for c in caps]
    cap_off = np.concatenate([[0], np.cumsum(caps)]).tolist()
    pad_edges = int(cap_off[-1])
    idx_cols = pad_edges // 16
    res_cols = pad_edges // 128

    def bucket_calls(cap):
        calls = []
        while cap > 0:
            c = min(cap, MAX_CALL)
            calls.append(c)
            cap -= c
        return calls

    bcalls = [bucket_calls(c) for c in caps]

    def bucket_batches(calls):
        batches, cur = [], []
        for c in calls:
            if cur and sum(cur) + c > MAX_BATCH_E:
                batches.append(cur)
                cur = []
            cur.append(c)
        if cur:
            batches.append(cur)
        return batches

    bbatch = [bucket_batches(c) for c in bcalls]
    max_batch = max(sum(b) for bb in bbatch for b in bb)

    batches = []  # (bucket, call list, edge offset within bucket)
    for bk in BUCKET_ORDER:
        off = 0
        for calls in bbatch[bk]:
            batches.append((bk, calls, off))
            off += sum(calls)

    gcum = [0, 0]
    gthresh = []
    for bi, (bk, calls, off) in enumerate(batches):
        gcum[bi % 2] += 2 * len(calls)
        gthresh.append(gcum[bi % 2])

    red_at = {}
    for bi, (bk, calls, off) in enumerate(batches):
        red_at[bk] = bi + 1

    # idx pieces needed before a bucket's gathers can run (pieces load in
    # a/b pairs; threshold counts both sides)
    piece_cols = idx_cols // IDX_DMA_PIECES
    idx_gate = {}
    for bk in range(NBUCKET):
        end_col = (cap_off[bk] + caps[bk]) // 16
        idx_gate[bk] = 32 * min(IDX_DMA_PIECES,
                                math.ceil(end_col / max(1, piece_cols)))

    return SimpleNamespace(
        caps=caps, cap_off=cap_off, pad_edges=pad_edges, idx_cols=idx_cols,
        res_cols=res_cols, bcalls=bcalls, bbatch=bbatch, max_batch=max_batch,
        batches=batches, nbatch=len(batches), gthresh=gthresh, red_at=red_at,
        piece_cols=piece_cols, idx_gate=idx_gate,
    )




# ---------------------------------------------------------------- device code
def build_bass(has_bias, sc):
    nc = bass.Bass()

    e1t = nc.dram_tensor("e1t", [D_IN, P1_ROWS], F16, kind="ExternalInput")
    e2t = nc.dram_tensor("e2t", [D_IN, P2_ROWS], F16, kind="ExternalInput")
    w1 = nc.dram_tensor("w1", [D_IN, D_OUT], F16, kind="ExternalInput")
    w2 = nc.dram_tensor("w2", [D_IN, D_OUT], F16, kind="ExternalInput")
    b1c = nc.dram_tensor("b1c", [1, D_OUT], F16, kind="ExternalInput")
    b2c = nc.dram_tensor("b2c", [1, D_OUT], F16, kind="ExternalInput")
    onesd = nc.dram_tensor("onesd", [1, 128], F16, kind="ExternalInput")
    idxa = nc.dram_tensor("idxa", [128, sc.idx_cols], I16, kind="ExternalInput")
    idxb = nc.dram_tensor("idxb", [128, sc.idx_cols], I16, kind="ExternalInput")
    res = nc.dram_tensor("res", [128, sc.res_cols], F16, kind="ExternalOutput")

    p1d = nc.dram_tensor("p1d", [P1_ROWS, D_OUT], F16, kind="Internal")
    p2d = nc.dram_tensor("p2d", [P2_ROWS, D_OUT], F16, kind="Internal")
    pdst = (p1d, p2d)
    pg = (p1d.bitcast(I32), p2d.bitcast(I32))
    poff = (F_OFF, T_OFF)

    per_group_mm = 12 if has_bias else 8
    mmc = [per_group_mm * (q + 1) for q in range(NGROUP)]

    st = contextlib.ExitStack()
    with st:
        sb = lambda nm, shape, dt=F16: st.enter_context(nc.sbuf_tensor(nm, shape, dt))
        sem = lambda nm: st.enter_context(nc.semaphore(name=nm))

        wc = (sb("w1c", [128, 256]), sb("w2c", [128, 256]))
        bc = (sb("b1s", [1, 128]), sb("b2s", [1, 128]))
        ones = sb("ones", [1, 128])
        idxt = (sb("idxta", [128, sc.idx_cols], I16),
                sb("idxtb", [128, sc.idx_cols], I16))
        et = [[[sb(f"et{tab}_{par}_{h}", [128, EMB_COLS]) for h in range(2)]
               for par in range(2)] for tab in range(2)]
        pv = [sb(f"pv{i}", [128, 512]) for i in range(4)]
        ps = [st.enter_context(nc.psum_tensor(f"ps{i}", [128, 512], F32))
              for i in range(4)]
        at = [sb(f"at{i}", [128, sc.max_batch]) for i in range(2)]
        btg = [sb(f"btg{i}", [128, sc.max_batch]) for i in range(2)]
        f1b = [sb(f"f1b{i}", [128, sc.max_batch // 2]) for i in range(2)]
        f2b = [sb(f"f2b{i}", [128, sc.max_batch // 4]) for i in range(2)]
        f3b = [sb(f"f3b{i}", [128, sc.max_batch // 8]) for i in range(2)]
        f4b = [sb(f"f4b{i}", [128, sc.max_batch // 16]) for i in range(2)]
        rt = [sb(f"rt{k}", [128, sc.caps[k] // 128]) for k in range(NBUCKET)]

        s_cl = sem("s_cl")
        s_ci = tuple(sem(f"s_ci{p}") for p in range(IDX_DMA_PIECES))
        s_ld = [tuple(sem(f"s_ld{t}_{p}") for p in range(2)) for t in range(2)]
        s_mm = sem("s_mm")
        s_dr = sem("s_dr")
        s_pw = tuple(sem(f"s_pw{i}") for i in range(4))
        s_g = tuple(sem(f"s_g{i}") for i in range(2))
        s_f1 = sem("s_f1")
        s_dv = sem("s_dv")
        s_red = sem("s_red")
        s_out = sem("s_out")

        BASE_CONSTS = 7 * 16  # w(4) + b(2) + ones(1)

        def pw_lane_counts(pos):
            return [len([q for q in range(pos) if q % 4 == r]) for r in range(4)]

        block = st.enter_context(nc.Block())

        # ------------------------------------------------ SP: loads + res out
        @block.sync
        def _(sync):
            for tab in range(2):
                w = (w1, w2)[tab]
                for k in range(2):
                    sync.dma_start(out=wc[tab][:, k * 128:(k + 1) * 128],
                                   in_=w[k * 128:(k + 1) * 128, :]).then_inc(s_cl, 16)
                sync.dma_start(out=bc[tab][:],
                               in_=(b1c, b2c)[tab][:]).then_inc(s_cl, 16)
            sync.dma_start(out=ones[:], in_=onesd[:]).then_inc(s_cl, 16)

            # emb chunks in first-use order (2-deep per table), with the idx
            # table uploads sliced in between the early chunk loads
            idx_piece = 0
            cnt = {0: 0, 1: 0}
            for ci, (tab, c) in enumerate(CSEQ):
                if cnt[tab] >= 2:
                    lastq = CHUNK_LAST[(tab, c - 2)]
                    sync.wait_ge(s_mm, mmc[lastq])
                src = (e1t, e2t)[tab]
                col0 = c * EMB_COLS
                par = cnt[tab] % 2
                sync.dma_start(out=et[tab][par][0][:],
                               in_=src[0:128, col0:col0 + EMB_COLS]).then_inc(
                    s_ld[tab][par], 16)
                sync.dma_start(out=et[tab][par][1][:],
                               in_=src[128:256, col0:col0 + EMB_COLS]).then_inc(
                    s_ld[tab][par], 16)
                cnt[tab] += 1
                if ci >= 1 and idx_piece < 2 * IDX_DMA_PIECES:
                    side, pc = idx_piece % 2, idx_piece // 2
                    c0 = pc * sc.piece_cols
                    cw = sc.piece_cols if pc < IDX_DMA_PIECES - 1 else sc.idx_cols - c0
                    sync.dma_start(out=idxt[side][:, c0:c0 + cw],
                                   in_=(idxa, idxb)[side][:, c0:c0 + cw]
                                   ).then_inc(s_ci[pc], 16)
                    idx_piece += 1
                if ci >= 1 and idx_piece < 2 * IDX_DMA_PIECES:
                    side, pc = idx_piece % 2, idx_piece // 2
                    c0 = pc * sc.piece_cols
                    cw = sc.piece_cols if pc < IDX_DMA_PIECES - 1 else sc.idx_cols - c0
                    sync.dma_start(out=idxt[side][:, c0:c0 + cw],
                                   in_=(idxa, idxb)[side][:, c0:c0 + cw]
                                   ).then_inc(s_ci[pc], 16)
                    idx_piece += 1

            for bk in BUCKET_ORDER:
                sync.wait_ge(s_red, sc.red_at[bk])
                k0 = sc.cap_off[bk] // 128
                sync.dma_start(out=res[:, k0:k0 + sc.caps[bk] // 128],
                               in_=rt[bk][:]).then_inc(s_out, 16)
            sync.wait_ge(s_out, 16 * NBUCKET)

        # p-write DMA for stream group q (reads pv[q%4], writes the p table);
        # issued from ACT for even q and from the PE (lagged) for odd q
        def emit_pwrite(eng, q):
            tab, g = GSEQ[q]
            eng.wait_ge(s_dr, q + 1)  # order the async DMA read
            r0 = g * 512
            eng.dma_start(
                out=pdst[tab][r0:r0 + 512, :].rearrange("(p j) d -> p (j d)", p=128),
                in_=pv[q % 4][:],
            ).then_inc(s_pw[q % 4], 16)

        # ------------------------------------------------ PE: projections
        @block.tensor
        def _(tensor):
            tensor.wait_ge(s_cl, BASE_CONSTS)
            for q, (tab, g) in enumerate(GSEQ):
                if q >= 4:
                    tensor.wait_ge(s_dr, q - 3)
                for j in range(4):
                    t = g * 4 + j
                    c = t // CHUNK_T
                    if t % CHUNK_T == 0:
                        tensor.wait_ge(s_ld[tab][c % 2], 32 * (c // 2 + 1))
                    out = ps[q % 4][:, j * 128:(j + 1) * 128]
                    col0 = (t % CHUNK_T) * 128
                    if has_bias:
                        tensor.matmul(out=out, lhsT=ones[:], rhs=bc[tab][:],
                                      start=True, stop=False).then_inc(s_mm, 1)
                    tensor.matmul(out=out, lhsT=et[tab][c % 2][0][:, col0:col0 + 128],
                                  rhs=wc[tab][:, 0:128], start=not has_bias,
                                  stop=False).then_inc(s_mm, 1)
                    tensor.matmul(out=out, lhsT=et[tab][c % 2][1][:, col0:col0 + 128],
                                  rhs=wc[tab][:, 128:256], start=False,
                                  stop=True).then_inc(s_mm, 1)


        # ------------------------------------------------ ACT: drains + even
        # p-writes (odd ones are issued from the PE to halve the ACT stream)
        @block.scalar
        def _(scalar):
            for q, (tab, g) in enumerate(GSEQ):
                if q < EARLY_DRAINS:
                    emit_pwrite(scalar, q)  # drain happens on the DVE
                    continue
                if q == EARLY_DRAINS:
                    scalar.wait_ge(s_dr, EARLY_DRAINS)  # order after DVE drains
                scalar.wait_ge(s_mm, mmc[q])
                if q >= 4:
                    scalar.wait_ge(s_pw[q % 4], 16 * (q // 4))
                scalar.copy(out=pv[q % 4][:], in_=ps[q % 4][:]).then_inc(s_dr, 1)
                emit_pwrite(scalar, q)

        # ------------------------------------------------ Pool: gathers
        @block.gpsimd
        def _(g_eng):
            from concourse import library_config
            g_eng.load_library(library_config.mlp)
            regs = {n: g_eng.to_reg(n) for n in
                    sorted({c for calls in sc.bcalls for c in calls})}

            g_eng.wait_ge(s_cl, BASE_CONSTS)
            gated = -1
            ci_gated = 0
            for bi, (bk, calls, off) in enumerate(sc.batches):
                fi, ti = bk // NT, bk % NT
                pos = BUCKET_POS[bk]
                if pos > gated:
                    for r, n in enumerate(pw_lane_counts(pos)):
                        if n:
                            g_eng.wait_ge(s_pw[r], 16 * n)
                    gated = pos
                need_pairs = sc.idx_gate[bk] // 32
                while ci_gated < need_pairs:
                    g_eng.wait_ge(s_ci[ci_gated], 32)
                    ci_gated += 1
                if bi >= 2:
                    g_eng.wait_ge(s_f1, bi - 1)  # at/btg[bi%2] consumed
                coff = 0
                for n in calls:
                    col0 = (sc.cap_off[bk] + off + coff) // 16
                    so = coff // 128
                    S = n // 128
                    for side, buf, tbl, pi in ((0, at, 0, fi), (1, btg, 1, ti)):
                        g_eng.dma_gather(
                            out_ap=buf[bi % 2][:, so * 128:(so + S) * 128]
                                .bitcast(I32).rearrange("p (s d) -> p s d", d=64),
                            in_ap=pg[tbl][poff[tbl][pi]:poff[tbl][pi + 1], :],
                            idxs_ap=idxt[side][:, col0:col0 + n // 16],
                            num_idxs=n, num_idxs_reg=regs[n], elem_size=64,
                            queue_num=0,
                        ).then_inc(s_g[bi % 2], 16)
                    coff += n

        # ------------------------------------------------ DVE: mul + folds + red
        @block.vector
        def _(vector):
            with nc.allow_low_precision(reason="fp16 dot products; 2e-2 tol"):
                for q in range(EARLY_DRAINS):
                    vector.wait_ge(s_mm, mmc[q])
                    if q >= 4:
                        vector.wait_ge(s_pw[q % 4], 16 * (q // 4))
                    vector.tensor_copy(out=pv[q % 4][:],
                                       in_=ps[q % 4][:]).then_inc(s_dr, 1)
                ndv = 0
                for bi, (bk, calls, off) in enumerate(sc.batches):
                    sz = sum(calls)
                    vector.wait_ge(s_g[bi % 2], 16 * sc.gthresh[bi])
                    a2 = at[bi % 2][:, :sz]
                    vector.tensor_mul(out=a2, in0=a2,
                                      in1=btg[bi % 2][:, :sz]).then_inc(s_dv, 1)
                    ndv += 1
                    a3 = at[bi % 2][:, :sz].rearrange("p (s d) -> p s d", d=128)
                    vector.wait_ge(s_dv, ndv)
                    vector.tensor_add(
                        out=f1b[bi % 2][:, :sz // 2].rearrange("p (s d) -> p s d", d=64),
                        in0=a3[:, :, 0:64], in1=a3[:, :, 64:128],
                    ).then_inc(s_f1, 1)
                    f1v = f1b[bi % 2][:, :sz // 2].rearrange("p (s d) -> p s d", d=64)
                    vector.wait_ge(s_f1, bi + 1)
                    vector.tensor_add(
                        out=f2b[bi % 2][:, :sz // 4].rearrange("p (s d) -> p s d", d=32),
                        in0=f1v[:, :, 0:32], in1=f1v[:, :, 32:64],
                    ).then_inc(s_dv, 1)
                    ndv += 1
                    f2v = f2b[bi % 2][:, :sz // 4].rearrange("p (s d) -> p s d", d=32)
                    vector.wait_ge(s_dv, ndv)
                    vector.tensor_add(
                        out=f3b[bi % 2][:, :sz // 8].rearrange("p (s d) -> p s d", d=16),
                        in0=f2v[:, :, 0:16], in1=f2v[:, :, 16:32],
                    ).then_inc(s_dv, 1)
                    ndv += 1
                    f3v = f3b[bi % 2][:, :sz // 8].rearrange("p (s d) -> p s d", d=16)
                    vector.wait_ge(s_dv, ndv)
                    vector.tensor_add(
                        out=f4b[bi % 2][:, :sz // 16].rearrange("p (s d) -> p s d", d=8),
                        in0=f3v[:, :, 0:8], in1=f3v[:, :, 8:16],
                    ).then_inc(s_dv, 1)
                    ndv += 1
                    vector.wait_ge(s_dv, ndv)
                    so = off // 128
                    vector.reduce_sum(
                        out=rt[bk][:, so:so + sz // 128],
                        in_=f4b[bi % 2][:, :sz // 16].rearrange("p (s d) -> p s d", d=8),
                        axis=AX.X,
                    ).then_inc(s_red, 1)

    return nc


_NC_CACHE = {}
_SCHED_CACHE = {}


def _get_sched(caps):
    key = tuple(caps)
    if key not in _SCHED_CACHE:
        _SCHED_CACHE[key] = make_sched(list(caps))
    return _SCHED_CACHE[key]


def _get_nc(has_bias, sc):
    key = (has_bias, tuple(sc.caps))
    if key not in _NC_CACHE:
        nc = build_bass(has_bias, sc)
        from concourse.library_overlay import lower_extended_insts
        lower_extended_insts(nc)
        _NC_CACHE[key] = nc
    return _NC_CACHE[key]


# ---------------------------------------------------------------- host side
def _to_f16(x):
    return np.ascontiguousarray(np.asarray(x, dtype=np.float32)).astype(np.float16)


def _marshal(emb_1, emb_2, nodes_from_to, W1, b1, W2, b2):
    """Shard/bucket inputs per core.  Returns (in_maps, books, has_bias, sc)."""
    f = np.asarray(nodes_from_to[:, 0], dtype=np.int64)
    t = np.asarray(nodes_from_to[:, 1], dtype=np.int64)
    emb_1 = np.asarray(emb_1, dtype=np.float32)
    emb_2 = np.asarray(emb_2, dtype=np.float32)
    W1h = _to_f16(W1)
    W2h = _to_f16(W2)
    b1v = np.asarray(b1, dtype=np.float32).reshape(-1)
    b2v = np.asarray(b2, dtype=np.float32).reshape(-1)
    has_bias = bool(b1v.any() or b2v.any())
    b1h = b1v.astype(np.float16).reshape(1, D_OUT)
    b2h = b2v.astype(np.float16).reshape(1, D_OUT)
    onesh = np.ones((1, 128), np.float16)

    core = (f // FSPAN) * 4 + t // TSPAN
    order0 = np.argsort(core, kind="stable")
    ccnt = np.bincount(core, minlength=N_CORES)
    coff = np.concatenate([[0], np.cumsum(ccnt)])

    # column-permuted transposed embedding shards:
    #   e_t[:, tile*128 + p] = emb_local[(tile//4)*512 + 4*p + (tile%4)]
    def build_embt(emb, lo, span, rows):
        ntile = rows // 128
        tiles = np.arange(ntile)
        cols_row = ((tiles[:, None] // 4) * 512 + 4 * np.arange(128)[None, :]
                    + (tiles[:, None] % 4)).reshape(-1)  # table row per column
        out = np.zeros((D_IN, rows), np.float16)
        valid = cols_row < span
        src = emb[lo:lo + span]
        out[:, valid] = src[cols_row[valid]].T.astype(np.float16)
        return out

    f_off = np.asarray(F_OFF)
    t_off = np.asarray(T_OFF)

    # bucket counts per core to pick the capacity tier
    percore = []
    for c in range(N_CORES):
        a, b = c // 4, c % 4
        sel = order0[coff[c]:coff[c + 1]]
        fl = f[sel] - a * FSPAN
        tl = t[sel] - b * TSPAN
        fi = np.searchsorted(f_off, fl, side="right") - 1
        ti = np.searchsorted(t_off, tl, side="right") - 1
        bk = fi * NT + ti
        o2 = np.argsort(bk, kind="stable")
        percore.append((sel[o2], bk[o2],
                        np.bincount(bk, minlength=NBUCKET)))

    maxcnt = np.max([p[2] for p in percore], axis=0)
    caps = [int(math.ceil(max(128, c) / 128) * 128) for c in maxcnt]
    sc = _get_sched(caps)

    in_maps, books = [], []
    for c in range(N_CORES):
        a, b = c // 4, c % 4
        sel2, bk2, cnts = percore[c]
        fl2 = f[sel2] - a * FSPAN
        tl2 = t[sel2] - b * TSPAN
        pos = np.concatenate([[0], np.cumsum(cnts)])

        ia = np.zeros(sc.pad_edges, np.int16)
        ib = np.zeros(sc.pad_edges, np.int16)
        for k in range(NBUCKET):
            i_, j_ = k // NT, k % NT
            n = cnts[k]
            o = sc.cap_off[k]
            ia[o:o + n] = (fl2[pos[k]:pos[k + 1]] - F_OFF[i_]).astype(np.int16)
            ib[o:o + n] = (tl2[pos[k]:pos[k + 1]] - T_OFF[j_]).astype(np.int16)
        # wrap by 16 per gather call; replicate content across partition groups
        wrapped_a = np.zeros((128, sc.idx_cols), np.int16)
        wrapped_b = np.zeros((128, sc.idx_cols), np.int16)
        for k in range(NBUCKET):
            o = sc.cap_off[k]
            coffs = 0
            for n in sc.bcalls[k]:
                seg_a = ia[o + coffs:o + coffs + n].reshape(n // 16, 16).T
                seg_b = ib[o + coffs:o + coffs + n].reshape(n // 16, 16).T
                c0 = (o + coffs) // 16
                wrapped_a[:, c0:c0 + n // 16] = np.tile(seg_a, (8, 1))
                wrapped_b[:, c0:c0 + n // 16] = np.tile(seg_b, (8, 1))
                coffs += n

        in_maps.append({
            "e1t": build_embt(emb_1, a * FSPAN, FSPAN, P1_ROWS),
            "e2t": build_embt(emb_2, b * TSPAN, TSPAN, P2_ROWS),
            "w1": W1h, "w2": W2h, "b1c": b1h, "b2c": b2h, "onesd": onesh,
            "idxa": wrapped_a, "idxb": wrapped_b,
        })
        books.append((sel2, cnts, pos))
    return in_maps, books, has_bias, sc


def _unmarshal(results, books, n_edges, sc):
    out = np.empty(n_edges, np.float32)
    for c in range(N_CORES):
        sel2, cnts, pos = books[c]
        r = results[c]["res"]  # [128, res_cols] f16
        vals = np.asarray(r).astype(np.float32).T.reshape(-1)  # e = s*128+p
        for k in range(NBUCKET):
            n = cnts[k]
            if n == 0:
                continue
            o = sc.cap_off[k]
            out[sel2[pos[k]:pos[k + 1]]] = vals[o:o + n]
    return out


def _run(inputs, trace=False, **run_kwargs):
    from concourse.bass_utils import run_bass_kernel_spmd

    in_maps, books, has_bias, sc = _marshal(**inputs)
    nc = _get_nc(has_bias, sc)
    r = run_bass_kernel_spmd(
        nc, in_maps, core_ids=list(range(N_CORES)), trace=trace, **run_kwargs
    )
    out = _unmarshal(r.results, books, len(inputs["nodes_from_to"]), sc)
    return out, r


def kernel(**inputs) -> np.ndarray:
    out, _ = _run(inputs, trace=False)
    return out


# revision 34
# speedup vs baseline: 2.5745x; 1.0792x over previous
"""Trainium2 Bass kernel for nn_BetweenClusterFC.

Computes out[e] = (emb_1[f[e]] @ W1 + b1) . (emb_2[t[e]] @ W2 + b2)
for E = 1.6M edges over N = 100k nodes, D_IN = 256, D_OUT = 128.

Strategy (8 NeuronCores, SPMD, full inputs in / full output out):
  - Edges are assigned to cores by a (from-half, to-quarter) 2x4 rectangle:
    core c=(a,b) handles edges with from-node in [50000a, 50000(a+1)) and
    to-node in [25000b, 25000(b+1)).  Each core projects its 75k nodes and
    stores p1/p2 fp16 row tables in local DRAM (tolerance is 2e-2; fp16
    keeps the end-to-end error ~7e-4).
  - The PE projects 128-row tiles (2 matmuls per tile, contraction 256 =
    2x128, optional per-tile bias matmul); the ACT engine drains PSUM to
    fp16 and issues the p-table writes, keeping the DVE free for dots.
  - The embedding shards are host-transposed AND column-permuted so each
    PSUM drain maps to one contiguous 1024-byte DRAM run per partition
    (rows 4p+j of each 512-row group live on partition p).
  - Edges are bucketed by (from-piece, to-piece), pieces being
    chunk-aligned slices of the node tables; the projection stream
    alternates to/from pieces (small pieces first) so the first buckets
    unlock ~15us in, and the DVE drains the first EARLY_DRAINS PSUM
    groups while it would otherwise idle.  Endpoint rows are fetched
    with SWDGE dma_gather reading the tables as int32[64] rows (raw
    256-byte row moves); dots are a DVE fp16 multiply + 4-level fold
    tree + reduce.
  - Bucket capacities are exact-fit to the input's bucket counts
    (rounded up to 128); the compiled program is cached per capacity
    vector, so repeated runs on the same inputs reuse the build.
  - The host applies the inverse edge permutation to assemble the output.

Written in raw Bass (explicit semaphores).
"""

import contextlib
import math
from types import SimpleNamespace

import numpy as np

import concourse.bass as bass
import concourse.mybir as mybir

# ---------------------------------------------------------------- constants
N_NODES = 100_000
D_IN = 256
D_OUT = 128
N_EDGES = 1_600_000
N_CORES = 8

FSPAN = 50_000        # from-node span per core (2 groups)
TSPAN = 25_000        # to-node span per core (4 groups)
P1_ROWS = 50_176      # padded from-table rows (98 groups of 512)
P2_ROWS = 25_088      # padded to-table rows (49 groups)
G1 = P1_ROWS // 512   # 98
G2 = P2_ROWS // 512   # 49
NGROUP = G1 + G2      # 147

CHUNK_T = 14                  # tiles per embT load chunk
EMB_COLS = CHUNK_T * 128      # 1792
NCH1 = P1_ROWS // EMB_COLS    # 28 chunks, from table
NCH2 = P2_ROWS // EMB_COLS    # 14 chunks, to table

# Node pieces (chunk-aligned, small first).  8 x 4 pieces -> 32 buckets.
F_PIECE_CH = [2, 3, 3, 4, 4, 4, 4, 4]
T_PIECE_CH = [2, 3, 4, 5]
NF = len(F_PIECE_CH)
NT = len(T_PIECE_CH)
NBUCKET = NF * NT

F_OFF = (np.concatenate([[0], np.cumsum(F_PIECE_CH)]) * EMB_COLS).tolist()
T_OFF = (np.concatenate([[0], np.cumsum(T_PIECE_CH)]) * EMB_COLS).tolist()
F_GROUPS = [math.ceil(F_OFF[i + 1] / 512) for i in range(NF)]
T_GROUPS = [math.ceil(T_OFF[i + 1] / 512) for i in range(NT)]

MAX_CALL = 1024     # idxs per dma_gather call (HW limit: <=1024 idxs/call)
MAX_BATCH_E = 4608  # edges per DVE batch
EARLY_DRAINS = 44   # PSUM drains done by the (initially idle) DVE
# drains completed before DVE starts batch bi (rest follow the last entry)
DRAIN_BEFORE_BATCH = {0: EARLY_DRAINS}
IDX_DMA_PIECES = 6  # idx table upload pieces per side (interleaved w/ chunks)
MARGIN_TIERS = (3.5, 8.0)  # bucket-cap sigma margins (tier 1 = fallback)

F32 = mybir.dt.float32
F16 = mybir.dt.float16
I16 = mybir.dt.int16
I32 = mybir.dt.int32
AX = mybir.AxisListType

# ------------------------------------------------ projection stream order
PIECE_STREAM = []
for _k in range(max(NF, NT)):
    if _k < NT:
        PIECE_STREAM.append((1, _k))  # tab 1 = to/p2
    if _k < NF:
        PIECE_STREAM.append((0, _k))  # tab 0 = from/p1

GSEQ = []  # [(tab, group)]
_done = {0: 0, 1: 0}
for _tab, _pc in PIECE_STREAM:
    _end = F_GROUPS[_pc] if _tab == 0 else T_GROUPS[_pc]
    for _g in range(_done[_tab], _end):
        GSEQ.append((_tab, _g))
    _done[_tab] = _end
assert len(GSEQ) == NGROUP
GPOS = {tg: q for q, tg in enumerate(GSEQ)}

PIECE_POS = {}
for _tab, _npc, _pg in ((0, NF, F_GROUPS), (1, NT, T_GROUPS)):
    for _pc in range(_npc):
        PIECE_POS[(_tab, _pc)] = GPOS[(_tab, _pg[_pc] - 1)] + 1

BUCKET_POS = {}
for _i in range(NF):
    for _j in range(NT):
        BUCKET_POS[_i * NT + _j] = max(PIECE_POS[(0, _i)], PIECE_POS[(1, _j)])
BUCKET_ORDER = sorted(range(NBUCKET), key=lambda b: (BUCKET_POS[b], b))

# chunk order by first use in the stream
_first = {}
for _q, (_tab, _g) in enumerate(GSEQ):
    for _c in range((_g * 4) // CHUNK_T, (_g * 4 + 3) // CHUNK_T + 1):
        _first.setdefault((_tab, _c), _q)
CSEQ = sorted(_first, key=lambda tc: _first[tc])
assert len(CSEQ) == NCH1 + NCH2

CHUNK_LAST = {}
for _q, (_tab, _g) in enumerate(GSEQ):
    for _c in range((_g * 4) // CHUNK_T, (_g * 4 + 3) // CHUNK_T + 1):
        CHUNK_LAST[(_tab, _c)] = _q


def margin_caps(margin):
    """Formula capacities: mean + margin sigma (used when no counts known)."""
    mean_edges = N_EDGES / N_CORES
    caps = []
    for i in range(NF):
        fr = min(F_OFF[i + 1], FSPAN) - min(F_OFF[i], FSPAN)
        for j in range(NT):
            tr = min(T_OFF[j + 1], TSPAN) - min(T_OFF[j], TSPAN)
            mean = mean_edges * (fr / FSPAN) * (tr / TSPAN)
            cap = mean + margin * math.sqrt(mean) + 64
            caps.append(int(math.ceil(cap / 128) * 128))
    return caps


def make_sched(caps):
    """Call/batch/gather schedule for explicit bucket capacities."""
    caps = [max(128, c) for c in caps]
    cap_off = np.concatenate([[0], np.cumsum(caps)]).tolist()
    pad_edges = int(cap_off[-1])
    idx_cols = pad_edges // 16
    res_cols = pad_edges // 128

    def bucket_calls(cap):
        calls = []
        while cap > 0:
            c = min(cap, MAX_CALL)
            calls.append(c)
            cap -= c
        return calls

    bcalls = [bucket_calls(c) for c in caps]

    def bucket_batches(calls):
        batches, cur = [], []
        for c in calls:
            if cur and sum(cur) + c > MAX_BATCH_E:
                batches.append(cur)
                cur = []
            cur.append(c)
        if cur:
            batches.append(cur)
        return batches

    bbatch = [bucket_batches(c) for c in bcalls]
    max_batch = max(sum(b) for bb in bbatch for b in bb)

    batches = []  # (bucket, call list, edge offset within bucket)
    for bk in BUCKET_ORDER:
        off = 0
        for calls in bbatch[bk]:
            batches.append((bk, calls, off))
            off += sum(calls)

    gcum = [0, 0]
    gthresh = []
    for bi, (bk, calls, off) in enumerate(batches):
        gcum[bi % 2] += 2 * len(calls)
        gthresh.append(gcum[bi % 2])

    red_at = {}
    for bi, (bk, calls, off) in enumerate(batches):
        red_at[bk] = bi + 1

    # idx pieces needed before a bucket's gathers can run (pieces load in
    # a/b pairs; threshold counts both sides)
    piece_cols = idx_cols // IDX_DMA_PIECES
    idx_gate = {}
    for bk in range(NBUCKET):
        end_col = (cap_off[bk] + caps[bk]) // 16
        idx_gate[bk] = 32 * min(IDX_DMA_PIECES,
                                math.ceil(end_col / max(1, piece_cols)))

    return SimpleNamespace(
        caps=caps, cap_off=cap_off, pad_edges=pad_edges, idx_cols=idx_cols,
        res_cols=res_cols, bcalls=bcalls, bbatch=bbatch, max_batch=max_batch,
        batches=batches, nbatch=len(batches), gthresh=gthresh, red_at=red_at,
        piece_cols=piece_cols, idx_gate=idx_gate,
    )




# ---------------------------------------------------------------- device code
def build_bass(has_bias, sc):
    nc = bass.Bass()

    e1t = nc.dram_tensor("e1t", [D_IN, P1_ROWS], F16, kind="ExternalInput")
    e2t = nc.dram_tensor("e2t", [D_IN, P2_ROWS], F16, kind="ExternalInput")
    w1 = nc.dram_tensor("w1", [D_IN, D_OUT], F16, kind="ExternalInput")
    w2 = nc.dram_tensor("w2", [D_IN, D_OUT], F16, kind="ExternalInput")
    b1c = nc.dram_tensor("b1c", [1, D_OUT], F16, kind="ExternalInput")
    b2c = nc.dram_tensor("b2c", [1, D_OUT], F16, kind="ExternalInput")
    onesd = nc.dram_tensor("onesd", [1, 128], F16, kind="ExternalInput")
    idxa = nc.dram_tensor("idxa", [128, sc.idx_cols], I16, kind="ExternalInput")
    idxb = nc.dram_tensor("idxb", [128, sc.idx_cols], I16, kind="ExternalInput")
    res = nc.dram_tensor("res", [128, sc.res_cols], F16, kind="ExternalOutput")

    p1d = nc.dram_tensor("p1d", [P1_ROWS, D_OUT], F16, kind="Internal")
    p2d = nc.dram_tensor("p2d", [P2_ROWS, D_OUT], F16, kind="Internal")
    pdst = (p1d, p2d)
    pg = (p1d.bitcast(I32), p2d.bitcast(I32))
    poff = (F_OFF, T_OFF)

    per_group_mm = 12 if has_bias else 8
    mmc = [per_group_mm * (q + 1) for q in range(NGROUP)]

    st = contextlib.ExitStack()
    with st:
        sb = lambda nm, shape, dt=F16: st.enter_context(nc.sbuf_tensor(nm, shape, dt))
        sem = lambda nm: st.enter_context(nc.semaphore(name=nm))

        wc = (sb("w1c", [128, 256]), sb("w2c", [128, 256]))
        bc = (sb("b1s", [1, 128]), sb("b2s", [1, 128]))
        ones = sb("ones", [1, 128])
        idxt = (sb("idxta", [128, sc.idx_cols], I16),
                sb("idxtb", [128, sc.idx_cols], I16))
        et = [[[sb(f"et{tab}_{par}_{h}", [128, EMB_COLS]) for h in range(2)]
               for par in range(2)] for tab in range(2)]
        pv = [sb(f"pv{i}", [128, 512]) for i in range(4)]
        ps = [st.enter_context(nc.psum_tensor(f"ps{i}", [128, 512], F32))
              for i in range(4)]
        at = [sb(f"at{i}", [128, sc.max_batch]) for i in range(2)]
        btg = [sb(f"btg{i}", [128, sc.max_batch]) for i in range(2)]
        f1b = [sb(f"f1b{i}", [128, sc.max_batch // 2]) for i in range(2)]
        f2b = [sb(f"f2b{i}", [128, sc.max_batch // 4]) for i in range(2)]
        f3b = [sb(f"f3b{i}", [128, sc.max_batch // 8]) for i in range(2)]
        f4b = [sb(f"f4b{i}", [128, sc.max_batch // 16]) for i in range(2)]
        rt = [sb(f"rt{k}", [128, sc.caps[k] // 128]) for k in range(NBUCKET)]

        s_cl = sem("s_cl")
        s_ci = tuple(sem(f"s_ci{p}") for p in range(IDX_DMA_PIECES))
        s_ld = [tuple(sem(f"s_ld{t}_{p}") for p in range(2)) for t in range(2)]
        s_mm = sem("s_mm")
        s_dr = sem("s_dr")
        s_pw = tuple(sem(f"s_pw{i}") for i in range(4))
        s_g = tuple(sem(f"s_g{i}") for i in range(2))
        s_f1 = sem("s_f1")
        s_dv = sem("s_dv")
        s_red = sem("s_red")
        s_out = sem("s_out")

        BASE_CONSTS = 7 * 16  # w(4) + b(2) + ones(1)

        def pw_lane_counts(pos):
            return [len([q for q in range(pos) if q % 4 == r]) for r in range(4)]

        block = st.enter_context(nc.Block())

        # ------------------------------------------------ SP: loads + res out
        @block.sync
        def _(sync):
            for tab in range(2):
                w = (w1, w2)[tab]
                for k in range(2):
                    sync.dma_start(out=wc[tab][:, k * 128:(k + 1) * 128],
                                   in_=w[k * 128:(k + 1) * 128, :]).then_inc(s_cl, 16)
                sync.dma_start(out=bc[tab][:],
                               in_=(b1c, b2c)[tab][:]).then_inc(s_cl, 16)
            sync.dma_start(out=ones[:], in_=onesd[:]).then_inc(s_cl, 16)

            # emb chunks in first-use order (2-deep per table), with the idx
            # table uploads sliced in between the early chunk loads
            # idx piece pairs 0-1 are loaded by the DVE at t=0; the rest
            # load here once the early chunk crunch is over
            idx_slots = {14: 2, 16: 3, 18: 4, 20: 5}
            cnt = {0: 0, 1: 0}
            for ci, (tab, c) in enumerate(CSEQ):
                if cnt[tab] >= 2:
                    lastq = CHUNK_LAST[(tab, c - 2)]
                    sync.wait_ge(s_mm, mmc[lastq])
                src = (e1t, e2t)[tab]
                col0 = c * EMB_COLS
                par = cnt[tab] % 2
                sync.dma_start(out=et[tab][par][0][:],
                               in_=src[0:128, col0:col0 + EMB_COLS]).then_inc(
                    s_ld[tab][par], 16)
                sync.dma_start(out=et[tab][par][1][:],
                               in_=src[128:256, col0:col0 + EMB_COLS]).then_inc(
                    s_ld[tab][par], 16)
                cnt[tab] += 1
                pc = idx_slots.get(ci)
                if pc is not None and pc < IDX_DMA_PIECES:
                    c0 = pc * sc.piece_cols
                    cw = sc.piece_cols if pc < IDX_DMA_PIECES - 1 else sc.idx_cols - c0
                    for side in range(2):
                        sync.dma_start(out=idxt[side][:, c0:c0 + cw],
                                       in_=(idxa, idxb)[side][:, c0:c0 + cw]
                                       ).then_inc(s_ci[pc], 16)

            for bk in BUCKET_ORDER:
                sync.wait_ge(s_red, sc.red_at[bk])
                k0 = sc.cap_off[bk] // 128
                sync.dma_start(out=res[:, k0:k0 + sc.caps[bk] // 128],
                               in_=rt[bk][:]).then_inc(s_out, 16)
            sync.wait_ge(s_out, 16 * NBUCKET)

        # p-write DMA for stream group q (reads pv[q%4], writes the p table);
        # issued from ACT for even q and from the PE (lagged) for odd q
        def emit_pwrite(eng, q):
            tab, g = GSEQ[q]
            eng.wait_ge(s_dr, q + 1)  # order the async DMA read
            r0 = g * 512
            eng.dma_start(
                out=pdst[tab][r0:r0 + 512, :].rearrange("(p j) d -> p (j d)", p=128),
                in_=pv[q % 4][:],
            ).then_inc(s_pw[q % 4], 16)

        # ------------------------------------------------ PE: projections
        @block.tensor
        def _(tensor):
            tensor.wait_ge(s_cl, BASE_CONSTS)
            for q, (tab, g) in enumerate(GSEQ):
                if q >= 4:
                    tensor.wait_ge(s_dr, q - 3)
                for j in range(4):
                    t = g * 4 + j
                    c = t // CHUNK_T
                    if t % CHUNK_T == 0:
                        tensor.wait_ge(s_ld[tab][c % 2], 32 * (c // 2 + 1))
                    out = ps[q % 4][:, j * 128:(j + 1) * 128]
                    col0 = (t % CHUNK_T) * 128
                    if has_bias:
                        tensor.matmul(out=out, lhsT=ones[:], rhs=bc[tab][:],
                                      start=True, stop=False).then_inc(s_mm, 1)
                    tensor.matmul(out=out, lhsT=et[tab][c % 2][0][:, col0:col0 + 128],
                                  rhs=wc[tab][:, 0:128], start=not has_bias,
                                  stop=False).then_inc(s_mm, 1)
                    tensor.matmul(out=out, lhsT=et[tab][c % 2][1][:, col0:col0 + 128],
                                  rhs=wc[tab][:, 128:256], start=False,
                                  stop=True).then_inc(s_mm, 1)


        # ------------------------------------------------ ACT: drains + even
        # p-writes (odd ones are issued from the PE to halve the ACT stream)
        @block.scalar
        def _(scalar):
            for pc in range(2):  # first idx pairs, before the drain stream
                c0 = pc * sc.piece_cols
                for side in range(2):
                    scalar.dma_start(out=idxt[side][:, c0:c0 + sc.piece_cols],
                                     in_=(idxa, idxb)[side][:, c0:c0 + sc.piece_cols]
                                     ).then_inc(s_ci[pc], 16)
            for q, (tab, g) in enumerate(GSEQ):
                if q < EARLY_DRAINS:
                    emit_pwrite(scalar, q)  # drain happens on the DVE
                    continue
                if q == EARLY_DRAINS:
                    scalar.wait_ge(s_dr, EARLY_DRAINS)  # order after DVE drains
                scalar.wait_ge(s_mm, mmc[q])
                if q >= 4:
                    scalar.wait_ge(s_pw[q % 4], 16 * (q // 4))
                scalar.copy(out=pv[q % 4][:], in_=ps[q % 4][:]).then_inc(s_dr, 1)
                emit_pwrite(scalar, q)

        # ------------------------------------------------ Pool: gathers
        @block.gpsimd
        def _(g_eng):
            from concourse import library_config
            g_eng.load_library(library_config.mlp)
            regs = {n: g_eng.to_reg(n) for n in
                    sorted({c for calls in sc.bcalls for c in calls})}

            g_eng.wait_ge(s_cl, BASE_CONSTS)
            gated = -1
            ci_gated = 0
            for bi, (bk, calls, off) in enumerate(sc.batches):
                fi, ti = bk // NT, bk % NT
                pos = BUCKET_POS[bk]
                if pos > gated:
                    for r, n in enumerate(pw_lane_counts(pos)):
                        if n:
                            g_eng.wait_ge(s_pw[r], 16 * n)
                    gated = pos
                need_pairs = sc.idx_gate[bk] // 32
                while ci_gated < need_pairs:
                    g_eng.wait_ge(s_ci[ci_gated], 32)
                    ci_gated += 1
                if bi >= 2:
                    g_eng.wait_ge(s_f1, bi - 1)  # at/btg[bi%2] consumed
                coff = 0
                for n in calls:
                    col0 = (sc.cap_off[bk] + off + coff) // 16
                    so = coff // 128
                    S = n // 128
                    for side, buf, tbl, pi in ((0, at, 0, fi), (1, btg, 1, ti)):
                        g_eng.dma_gather(
                            out_ap=buf[bi % 2][:, so * 128:(so + S) * 128]
                                .bitcast(I32).rearrange("p (s d) -> p s d", d=64),
                            in_ap=pg[tbl][poff[tbl][pi]:poff[tbl][pi + 1], :],
                            idxs_ap=idxt[side][:, col0:col0 + n // 16],
                            num_idxs=n, num_idxs_reg=regs[n], elem_size=64,
                            queue_num=0,
                        ).then_inc(s_g[bi % 2], 16)
                    coff += n

        # ------------------------------------------------ DVE: mul + folds + red
        @block.vector
        def _(vector):
            with nc.allow_low_precision(reason="fp16 dot products; 2e-2 tol"):
                def drain(q):
                    vector.wait_ge(s_mm, mmc[q])
                    if q >= 4:
                        vector.wait_ge(s_pw[q % 4], 16 * (q // 4))
                    vector.tensor_copy(out=pv[q % 4][:],
                                       in_=ps[q % 4][:]).then_inc(s_dr, 1)

                # early drains run before the batch loop; interleaving
                # batches into the chain was tried and stalls the projection
                # front (DRAIN_BEFORE_BATCH kept for experimentation)
                drained = 0
                ndv = 0
                for bi, (bk, calls, off) in enumerate(sc.batches):
                    target = DRAIN_BEFORE_BATCH.get(bi, EARLY_DRAINS if bi > 4
                                                    else EARLY_DRAINS)
                    while drained < target:
                        drain(drained)
                        drained += 1
                    sz = sum(calls)
                    vector.wait_ge(s_g[bi % 2], 16 * sc.gthresh[bi])
                    a2 = at[bi % 2][:, :sz]
                    vector.tensor_mul(out=a2, in0=a2,
                                      in1=btg[bi % 2][:, :sz]).then_inc(s_dv, 1)
                    ndv += 1
                    a3 = at[bi % 2][:, :sz].rearrange("p (s d) -> p s d", d=128)
                    vector.wait_ge(s_dv, ndv)
                    vector.tensor_add(
                        out=f1b[bi % 2][:, :sz // 2].rearrange("p (s d) -> p s d", d=64),
                        in0=a3[:, :, 0:64], in1=a3[:, :, 64:128],
                    ).then_inc(s_f1, 1)
                    f1v = f1b[bi % 2][:, :sz // 2].rearrange("p (s d) -> p s d", d=64)
                    vector.wait_ge(s_f1, bi + 1)
                    vector.tensor_add(
                        out=f2b[bi % 2][:, :sz // 4].rearrange("p (s d) -> p s d", d=32),
                        in0=f1v[:, :, 0:32], in1=f1v[:, :, 32:64],
                    ).then_inc(s_dv, 1)
                    ndv += 1
                    f2v = f2b[bi % 2][:, :sz // 4].rearrange("p (s d) -> p s d", d=32)
                    vector.wait_ge(s_dv, ndv)
                    vector.tensor_add(
                        out=f3b[bi % 2][:, :sz // 8].rearrange("p (s d) -> p s d", d=16),
                        in0=f2v[:, :, 0:16], in1=f2v[:, :, 16:32],
                    ).then_inc(s_dv, 1)
                    ndv += 1
                    f3v = f3b[bi % 2][:, :sz // 8].rearrange("p (s d) -> p s d", d=16)
                    vector.wait_ge(s_dv, ndv)
                    vector.tensor_add(
                        out=f4b[bi % 2][:, :sz // 16].rearrange("p (s d) -> p s d", d=8),
                        in0=f3v[:, :, 0:8], in1=f3v[:, :, 8:16],
                    ).then_inc(s_dv, 1)
                    ndv += 1
                    vector.wait_ge(s_dv, ndv)
                    so = off // 128
                    vector.reduce_sum(
                        out=rt[bk][:, so:so + sz // 128],
                        in_=f4b[bi % 2][:, :sz // 16].rearrange("p (s d) -> p s d", d=8),
                        axis=AX.X,
                    ).then_inc(s_red, 1)

    return nc


_NC_CACHE = {}
_SCHED_CACHE = {}


def _get_sched(caps):
    key = tuple(caps)
    if key not in _SCHED_CACHE:
        _SCHED_CACHE[key] = make_sched(list(caps))
    return _SCHED_CACHE[key]


def _get_nc(has_bias, sc):
    key = (has_bias, tuple(sc.caps))
    if key not in _NC_CACHE:
        nc = build_bass(has_bias, sc)
        from concourse.library_overlay import lower_extended_insts
        lower_extended_insts(nc)
        _NC_CACHE[key] = nc
    return _NC_CACHE[key]


# ---------------------------------------------------------------- host side
def _to_f16(x):
    return np.ascontiguousarray(np.asarray(x, dtype=np.float32)).astype(np.float16)


def _marshal(emb_1, emb_2, nodes_from_to, W1, b1, W2, b2):
    """Shard/bucket inputs per core.  Returns (in_maps, books, has_bias, sc)."""
    f = np.asarray(nodes_from_to[:, 0], dtype=np.int64)
    t = np.asarray(nodes_from_to[:, 1], dtype=np.int64)
    emb_1 = np.asarray(emb_1, dtype=np.float32)
    emb_2 = np.asarray(emb_2, dtype=np.float32)
    W1h = _to_f16(W1)
    W2h = _to_f16(W2)
    b1v = np.asarray(b1, dtype=np.float32).reshape(-1)
    b2v = np.asarray(b2, dtype=np.float32).reshape(-1)
    has_bias = bool(b1v.any() or b2v.any())
    b1h = b1v.astype(np.float16).reshape(1, D_OUT)
    b2h = b2v.astype(np.float16).reshape(1, D_OUT)
    onesh = np.ones((1, 128), np.float16)

    core = (f // FSPAN) * 4 + t // TSPAN
    order0 = np.argsort(core, kind="stable")
    ccnt = np.bincount(core, minlength=N_CORES)
    coff = np.concatenate([[0], np.cumsum(ccnt)])

    # column-permuted transposed embedding shards:
    #   e_t[:, tile*128 + p] = emb_local[(tile//4)*512 + 4*p + (tile%4)]
    def build_embt(emb, lo, span, rows):
        ntile = rows // 128
        tiles = np.arange(ntile)
        cols_row = ((tiles[:, None] // 4) * 512 + 4 * np.arange(128)[None, :]
                    + (tiles[:, None] % 4)).reshape(-1)  # table row per column
        out = np.zeros((D_IN, rows), np.float16)
        valid = cols_row < span
        src = emb[lo:lo + span]
        out[:, valid] = src[cols_row[valid]].T.astype(np.float16)
        return out

    f_off = np.asarray(F_OFF)
    t_off = np.asarray(T_OFF)

    # bucket counts per core to pick the capacity tier
    percore = []
    for c in range(N_CORES):
        a, b = c // 4, c % 4
        sel = order0[coff[c]:coff[c + 1]]
        fl = f[sel] - a * FSPAN
        tl = t[sel] - b * TSPAN
        fi = np.searchsorted(f_off, fl, side="right") - 1
        ti = np.searchsorted(t_off, tl, side="right") - 1
        bk = fi * NT + ti
        o2 = np.argsort(bk, kind="stable")
        percore.append((sel[o2], bk[o2],
                        np.bincount(bk, minlength=NBUCKET)))

    maxcnt = np.max([p[2] for p in percore], axis=0)
    caps = [int(math.ceil(max(128, c) / 128) * 128) for c in maxcnt]
    sc = _get_sched(caps)

    in_maps, books = [], []
    for c in range(N_CORES):
        a, b = c // 4, c % 4
        sel2, bk2, cnts = percore[c]
        fl2 = f[sel2] - a * FSPAN
        tl2 = t[sel2] - b * TSPAN
        pos = np.concatenate([[0], np.cumsum(cnts)])

        ia = np.zeros(sc.pad_edges, np.int16)
        ib = np.zeros(sc.pad_edges, np.int16)
        for k in range(NBUCKET):
            i_, j_ = k // NT, k % NT
            n = cnts[k]
            o = sc.cap_off[k]
            ia[o:o + n] = (fl2[pos[k]:pos[k + 1]] - F_OFF[i_]).astype(np.int16)
            ib[o:o + n] = (tl2[pos[k]:pos[k + 1]] - T_OFF[j_]).astype(np.int16)
        # wrap by 16 per gather call; replicate content across partition groups
        wrapped_a = np.zeros((128, sc.idx_cols), np.int16)
        wrapped_b = np.zeros((128, sc.idx_cols), np.int16)
        for k in range(NBUCKET):
            o = sc.cap_off[k]
            coffs = 0
            for n in sc.bcalls[k]:
                seg_a = ia[o + coffs:o + coffs + n].reshape(n // 16, 16).T
                seg_b = ib[o + coffs:o + coffs + n].reshape(n // 16, 16).T
                c0 = (o + coffs) // 16
                wrapped_a[:, c0:c0 + n // 16] = np.tile(seg_a, (8, 1))
                wrapped_b[:, c0:c0 + n // 16] = np.tile(seg_b, (8, 1))
                coffs += n

        in_maps.append({
            "e1t": build_embt(emb_1, a * FSPAN, FSPAN, P1_ROWS),
            "e2t": build_embt(emb_2, b * TSPAN, TSPAN, P2_ROWS),
            "w1": W1h, "w2": W2h, "b1c": b1h, "b2c": b2h, "onesd": onesh,
            "idxa": wrapped_a, "idxb": wrapped_b,
        })
        books.append((sel2, cnts, pos))
    return in_maps, books, has_bias, sc


def _unmarshal(results, books, n_edges, sc):
    out = np.empty(n_edges, np.float32)
    for c in range(N_CORES):
        sel2, cnts, pos = books[c]
        r = results[c]["res"]  # [128, res_cols] f16
        vals = np.asarray(r).astype(np.float32).T.reshape(-1)  # e = s*128+p
        for k in range(NBUCKET):
            n = cnts[k]
            if n == 0:
                continue
            o = sc.cap_off[k]
            out[sel2[pos[k]:pos[k + 1]]] = vals[o:o + n]
    return out


def _run(inputs, trace=False, **run_kwargs):
    from concourse.bass_utils import run_bass_kernel_spmd

    in_maps, books, has_bias, sc = _marshal(**inputs)
    nc = _get_nc(has_bias, sc)
    r = run_bass_kernel_spmd(
        nc, in_maps, core_ids=list(range(N_CORES)), trace=trace, **run_kwargs
    )
    out = _unmarshal(r.results, books, len(inputs["nodes_from_to"]), sc)
    return out, r


def kernel(**inputs) -> np.ndarray:
    out, _ = _run(inputs, trace=False)
    return out


# revision 37
# speedup vs baseline: 2.5929x; 1.0071x over previous
"""Trainium2 Bass kernel for nn_BetweenClusterFC.

Computes out[e] = (emb_1[f[e]] @ W1 + b1) . (emb_2[t[e]] @ W2 + b2)
for E = 1.6M edges over N = 100k nodes, D_IN = 256, D_OUT = 128.

Strategy (8 NeuronCores, SPMD, full inputs in / full output out):
  - Edges are assigned to cores by a (from-half, to-quarter) 2x4 rectangle:
    core c=(a,b) handles edges with from-node in [50000a, 50000(a+1)) and
    to-node in [25000b, 25000(b+1)).  Each core projects its 75k nodes and
    stores p1/p2 fp16 row tables in local DRAM (tolerance is 2e-2; fp16
    keeps the end-to-end error ~7e-4).
  - The PE projects 128-row tiles (2 matmuls per tile, contraction 256 =
    2x128, optional per-tile bias matmul); the ACT engine drains PSUM to
    fp16 and issues the p-table writes, keeping the DVE free for dots.
  - The embedding shards are host-transposed AND column-permuted so each
    PSUM drain maps to one contiguous 1024-byte DRAM run per partition
    (rows 4p+j of each 512-row group live on partition p).
  - Edges are bucketed by (from-piece, to-piece), pieces being
    chunk-aligned slices of the node tables; the projection stream
    alternates to/from pieces (small pieces first) so the first buckets
    unlock ~15us in, and the DVE drains the first EARLY_DRAINS PSUM
    groups while it would otherwise idle.  Endpoint rows are fetched
    with SWDGE dma_gather reading the tables as int32[64] rows (raw
    256-byte row moves); dots are a DVE fp16 multiply + 4-level fold
    tree + reduce.
  - Bucket capacities are exact-fit to the input's bucket counts
    (rounded up to 128); the compiled program is cached per capacity
    vector, so repeated runs on the same inputs reuse the build.
  - The host applies the inverse edge permutation to assemble the output.

Written in raw Bass (explicit semaphores).
"""

import contextlib
import math
from types import SimpleNamespace

import numpy as np

import concourse.bass as bass
import concourse.mybir as mybir

# ---------------------------------------------------------------- constants
N_NODES = 100_000
D_IN = 256
D_OUT = 128
N_EDGES = 1_600_000
N_CORES = 8

FSPAN = 50_000        # from-node span per core (2 groups)
TSPAN = 25_000        # to-node span per core (4 groups)
P1_ROWS = 50_176      # padded from-table rows (98 groups of 512)
P2_ROWS = 25_088      # padded to-table rows (49 groups)
G1 = P1_ROWS // 512   # 98
G2 = P2_ROWS // 512   # 49
NGROUP = G1 + G2      # 147

CHUNK_T = 14                  # tiles per embT load chunk
EMB_COLS = CHUNK_T * 128      # 1792
NCH1 = P1_ROWS // EMB_COLS    # 28 chunks, from table
NCH2 = P2_ROWS // EMB_COLS    # 14 chunks, to table

# Node pieces (chunk-aligned, small first).  8 x 4 pieces -> 32 buckets.
F_PIECE_CH = [2, 3, 3, 4, 4, 4, 4, 4]
T_PIECE_CH = [2, 3, 4, 5]
NF = len(F_PIECE_CH)
NT = len(T_PIECE_CH)
NBUCKET = NF * NT

F_OFF = (np.concatenate([[0], np.cumsum(F_PIECE_CH)]) * EMB_COLS).tolist()
T_OFF = (np.concatenate([[0], np.cumsum(T_PIECE_CH)]) * EMB_COLS).tolist()
F_GROUPS = [math.ceil(F_OFF[i + 1] / 512) for i in range(NF)]
T_GROUPS = [math.ceil(T_OFF[i + 1] / 512) for i in range(NT)]

MAX_CALL = 1024     # idxs per dma_gather call (HW limit: <=1024 idxs/call)
MAX_BATCH_E = 4608  # edges per DVE batch
EARLY_DRAINS = 44   # PSUM drains done by the (initially idle) DVE
# drains completed before DVE starts batch bi (rest follow the last entry)
DRAIN_BEFORE_BATCH = {0: EARLY_DRAINS}
IDX_DMA_PIECES = 6  # idx table upload pieces per side (interleaved w/ chunks)
MARGIN_TIERS = (3.5, 8.0)  # bucket-cap sigma margins (tier 1 = fallback)

F32 = mybir.dt.float32
F16 = mybir.dt.float16
I16 = mybir.dt.int16
I32 = mybir.dt.int32
AX = mybir.AxisListType

# ------------------------------------------------ projection stream order
PIECE_STREAM = []
for _k in range(max(NF, NT)):
    if _k < NT:
        PIECE_STREAM.append((1, _k))  # tab 1 = to/p2
    if _k < NF:
        PIECE_STREAM.append((0, _k))  # tab 0 = from/p1

GSEQ = []  # [(tab, group)]
_done = {0: 0, 1: 0}
for _tab, _pc in PIECE_STREAM:
    _end = F_GROUPS[_pc] if _tab == 0 else T_GROUPS[_pc]
    for _g in range(_done[_tab], _end):
        GSEQ.append((_tab, _g))
    _done[_tab] = _end
assert len(GSEQ) == NGROUP
GPOS = {tg: q for q, tg in enumerate(GSEQ)}

PIECE_POS = {}
for _tab, _npc, _pg in ((0, NF, F_GROUPS), (1, NT, T_GROUPS)):
    for _pc in range(_npc):
        PIECE_POS[(_tab, _pc)] = GPOS[(_tab, _pg[_pc] - 1)] + 1

BUCKET_POS = {}
for _i in range(NF):
    for _j in range(NT):
        BUCKET_POS[_i * NT + _j] = max(PIECE_POS[(0, _i)], PIECE_POS[(1, _j)])
BUCKET_ORDER = sorted(range(NBUCKET), key=lambda b: (BUCKET_POS[b], b))

# chunk order by first use in the stream
_first = {}
for _q, (_tab, _g) in enumerate(GSEQ):
    for _c in range((_g * 4) // CHUNK_T, (_g * 4 + 3) // CHUNK_T + 1):
        _first.setdefault((_tab, _c), _q)
CSEQ = sorted(_first, key=lambda tc: _first[tc])
assert len(CSEQ) == NCH1 + NCH2

CHUNK_LAST = {}
for _q, (_tab, _g) in enumerate(GSEQ):
    for _c in range((_g * 4) // CHUNK_T, (_g * 4 + 3) // CHUNK_T + 1):
        CHUNK_LAST[(_tab, _c)] = _q


def margin_caps(margin):
    """Formula capacities: mean + margin sigma (used when no counts known)."""
    mean_edges = N_EDGES / N_CORES
    caps = []
    for i in range(NF):
        fr = min(F_OFF[i + 1], FSPAN) - min(F_OFF[i], FSPAN)
        for j in range(NT):
            tr = min(T_OFF[j + 1], TSPAN) - min(T_OFF[j], TSPAN)
            mean = mean_edges * (fr / FSPAN) * (tr / TSPAN)
            cap = mean + margin * math.sqrt(mean) + 64
            caps.append(int(math.ceil(cap / 128) * 128))
    return caps


def make_sched(caps):
    """Call/batch/gather schedule for explicit bucket capacities."""
    caps = [max(128, c) for c in caps]
    cap_off = np.concatenate([[0], np.cumsum(caps)]).tolist()
    pad_edges = int(cap_off[-1])
    idx_cols = pad_edges // 16
    res_cols = pad_edges // 128

    def bucket_calls(cap):
        calls = []
        while cap > 0:
            c = min(cap, MAX_CALL)
            calls.append(c)
            cap -= c
        return calls

    bcalls = [bucket_calls(c) for c in caps]

    def bucket_batches(calls):
        batches, cur = [], []
        for c in calls:
            if cur and sum(cur) + c > MAX_BATCH_E:
                batches.append(cur)
                cur = []
            cur.append(c)
        if cur:
            batches.append(cur)
        return batches

    bbatch = [bucket_batches(c) for c in bcalls]
    max_batch = max(sum(b) for bb in bbatch for b in bb)

    batches = []  # (bucket, call list, edge offset within bucket)
    for bk in BUCKET_ORDER:
        off = 0
        for calls in bbatch[bk]:
            batches.append((bk, calls, off))
            off += sum(calls)

    gcum = [0, 0]
    gthresh = []
    for bi, (bk, calls, off) in enumerate(batches):
        gcum[bi % 2] += 2 * len(calls)
        gthresh.append(gcum[bi % 2])

    red_at = {}
    for bi, (bk, calls, off) in enumerate(batches):
        red_at[bk] = bi + 1

    # idx pieces needed before a bucket's gathers can run (pieces load in
    # a/b pairs; threshold counts both sides)
    piece_cols = idx_cols // IDX_DMA_PIECES
    idx_gate = {}
    for bk in range(NBUCKET):
        end_col = (cap_off[bk] + caps[bk]) // 16
        idx_gate[bk] = 32 * min(IDX_DMA_PIECES,
                                math.ceil(end_col / max(1, piece_cols)))

    return SimpleNamespace(
        caps=caps, cap_off=cap_off, pad_edges=pad_edges, idx_cols=idx_cols,
        res_cols=res_cols, bcalls=bcalls, bbatch=bbatch, max_batch=max_batch,
        batches=batches, nbatch=len(batches), gthresh=gthresh, red_at=red_at,
        piece_cols=piece_cols, idx_gate=idx_gate,
    )




# ---------------------------------------------------------------- device code
def build_bass(has_bias, sc):
    nc = bass.Bass()

    e1t = nc.dram_tensor("e1t", [D_IN, P1_ROWS], F16, kind="ExternalInput")
    e2t = nc.dram_tensor("e2t", [D_IN, P2_ROWS], F16, kind="ExternalInput")
    w1 = nc.dram_tensor("w1", [D_IN, D_OUT], F16, kind="ExternalInput")
    w2 = nc.dram_tensor("w2", [D_IN, D_OUT], F16, kind="ExternalInput")
    b1c = nc.dram_tensor("b1c", [1, D_OUT], F16, kind="ExternalInput")
    b2c = nc.dram_tensor("b2c", [1, D_OUT], F16, kind="ExternalInput")
    onesd = nc.dram_tensor("onesd", [1, 128], F16, kind="ExternalInput")
    idxa = nc.dram_tensor("idxa", [128, sc.idx_cols], I16, kind="ExternalInput")
    idxb = nc.dram_tensor("idxb", [128, sc.idx_cols], I16, kind="ExternalInput")
    res = nc.dram_tensor("res", [128, sc.res_cols], F16, kind="ExternalOutput")

    p1d = nc.dram_tensor("p1d", [P1_ROWS, D_OUT], F16, kind="Internal")
    p2d = nc.dram_tensor("p2d", [P2_ROWS, D_OUT], F16, kind="Internal")
    pdst = (p1d, p2d)
    pg = (p1d.bitcast(I32), p2d.bitcast(I32))
    poff = (F_OFF, T_OFF)

    per_group_mm = 12 if has_bias else 8
    mmc = [per_group_mm * (q + 1) for q in range(NGROUP)]

    st = contextlib.ExitStack()
    with st:
        sb = lambda nm, shape, dt=F16: st.enter_context(nc.sbuf_tensor(nm, shape, dt))
        sem = lambda nm: st.enter_context(nc.semaphore(name=nm))

        wc = (sb("w1c", [128, 256]), sb("w2c", [128, 256]))
        bc = (sb("b1s", [1, 128]), sb("b2s", [1, 128]))
        ones = sb("ones", [1, 128])
        idxt = (sb("idxta", [128, sc.idx_cols], I16),
                sb("idxtb", [128, sc.idx_cols], I16))
        et = [[[sb(f"et{tab}_{par}_{h}", [128, EMB_COLS]) for h in range(2)]
               for par in range(2)] for tab in range(2)]
        pv = [sb(f"pv{i}", [128, 512]) for i in range(4)]
        ps = [st.enter_context(nc.psum_tensor(f"ps{i}", [128, 512], F32))
              for i in range(4)]
        at = [sb(f"at{i}", [128, sc.max_batch]) for i in range(2)]
        btg = [sb(f"btg{i}", [128, sc.max_batch]) for i in range(2)]
        f1b = [sb(f"f1b{i}", [128, sc.max_batch // 2]) for i in range(2)]
        f2b = [sb(f"f2b{i}", [128, sc.max_batch // 4]) for i in range(2)]
        f3b = [sb(f"f3b{i}", [128, sc.max_batch // 8]) for i in range(2)]
        f4b = [sb(f"f4b{i}", [128, sc.max_batch // 16]) for i in range(2)]
        rt = [sb(f"rt{k}", [128, sc.caps[k] // 128]) for k in range(NBUCKET)]

        s_cl = sem("s_cl")
        s_ci = tuple(sem(f"s_ci{p}") for p in range(IDX_DMA_PIECES))
        s_ld = [tuple(sem(f"s_ld{t}_{p}") for p in range(2)) for t in range(2)]
        s_mm = sem("s_mm")
        s_dr = sem("s_dr")
        s_pw = tuple(sem(f"s_pw{i}") for i in range(4))
        s_g = tuple(sem(f"s_g{i}") for i in range(2))
        s_f1 = sem("s_f1")
        s_dv = sem("s_dv")
        s_red = sem("s_red")
        s_out = sem("s_out")

        BASE_CONSTS = 7 * 16  # w(4) + b(2) + ones(1)

        def pw_lane_counts(pos):
            return [len([q for q in range(pos) if q % 4 == r]) for r in range(4)]

        block = st.enter_context(nc.Block())

        # ------------------------------------------------ SP: loads + res out
        @block.sync
        def _(sync):
            for tab in range(2):
                w = (w1, w2)[tab]
                for k in range(2):
                    sync.dma_start(out=wc[tab][:, k * 128:(k + 1) * 128],
                                   in_=w[k * 128:(k + 1) * 128, :]).then_inc(s_cl, 16)
                sync.dma_start(out=bc[tab][:],
                               in_=(b1c, b2c)[tab][:]).then_inc(s_cl, 16)
            sync.dma_start(out=ones[:], in_=onesd[:]).then_inc(s_cl, 16)

            # emb chunks in first-use order (2-deep per table), with the idx
            # table uploads sliced in between the early chunk loads
            # chunks (0,0) and (0,1) are loaded by the ACT engine at t=0
            # idx piece pairs 0-1 are loaded by the DVE at t=0; the rest
            # load here once the early chunk crunch is over
            idx_slots = {14: 2, 16: 3, 18: 4, 20: 5}
            cnt = {0: 2, 1: 0}  # ACT pre-loads from-table chunks 0 and 1
            for ci, (tab, c) in enumerate(CSEQ):
                if tab == 0 and c < 2:
                    continue
                if cnt[tab] >= 2:
                    lastq = CHUNK_LAST[(tab, c - 2)]
                    sync.wait_ge(s_mm, mmc[lastq])
                src = (e1t, e2t)[tab]
                col0 = c * EMB_COLS
                par = cnt[tab] % 2
                sync.dma_start(out=et[tab][par][0][:],
                               in_=src[0:128, col0:col0 + EMB_COLS]).then_inc(
                    s_ld[tab][par], 16)
                sync.dma_start(out=et[tab][par][1][:],
                               in_=src[128:256, col0:col0 + EMB_COLS]).then_inc(
                    s_ld[tab][par], 16)
                cnt[tab] += 1
                pc = idx_slots.get(ci)
                if pc is not None and pc < IDX_DMA_PIECES:
                    c0 = pc * sc.piece_cols
                    cw = sc.piece_cols if pc < IDX_DMA_PIECES - 1 else sc.idx_cols - c0
                    for side in range(2):
                        sync.dma_start(out=idxt[side][:, c0:c0 + cw],
                                       in_=(idxa, idxb)[side][:, c0:c0 + cw]
                                       ).then_inc(s_ci[pc], 16)

            for bk in BUCKET_ORDER:
                sync.wait_ge(s_red, sc.red_at[bk])
                k0 = sc.cap_off[bk] // 128
                sync.dma_start(out=res[:, k0:k0 + sc.caps[bk] // 128],
                               in_=rt[bk][:]).then_inc(s_out, 16)
            sync.wait_ge(s_out, 16 * NBUCKET)

        # p-write DMA for stream group q (reads pv[q%4], writes the p table);
        # issued from ACT for even q and from the PE (lagged) for odd q
        def emit_pwrite(eng, q):
            tab, g = GSEQ[q]
            eng.wait_ge(s_dr, q + 1)  # order the async DMA read
            r0 = g * 512
            eng.dma_start(
                out=pdst[tab][r0:r0 + 512, :].rearrange("(p j) d -> p (j d)", p=128),
                in_=pv[q % 4][:],
            ).then_inc(s_pw[q % 4], 16)

        # ------------------------------------------------ PE: projections
        @block.tensor
        def _(tensor):
            tensor.wait_ge(s_cl, BASE_CONSTS)
            for q, (tab, g) in enumerate(GSEQ):
                if q >= 4:
                    tensor.wait_ge(s_dr, q - 3)
                for j in range(4):
                    t = g * 4 + j
                    c = t // CHUNK_T
                    if t % CHUNK_T == 0:
                        tensor.wait_ge(s_ld[tab][c % 2], 32 * (c // 2 + 1))
                    out = ps[q % 4][:, j * 128:(j + 1) * 128]
                    col0 = (t % CHUNK_T) * 128
                    if has_bias:
                        tensor.matmul(out=out, lhsT=ones[:], rhs=bc[tab][:],
                                      start=True, stop=False).then_inc(s_mm, 1)
                    tensor.matmul(out=out, lhsT=et[tab][c % 2][0][:, col0:col0 + 128],
                                  rhs=wc[tab][:, 0:128], start=not has_bias,
                                  stop=False).then_inc(s_mm, 1)
                    tensor.matmul(out=out, lhsT=et[tab][c % 2][1][:, col0:col0 + 128],
                                  rhs=wc[tab][:, 128:256], start=False,
                                  stop=True).then_inc(s_mm, 1)


        # ------------------------------------------------ ACT: drains + even
        # p-writes (odd ones are issued from the PE to halve the ACT stream)
        @block.scalar
        def _(scalar):
            for c in range(2):  # first two from-table chunks, in parallel w/ SP
                col0 = c * EMB_COLS
                for h in range(2):
                    scalar.dma_start(out=et[0][c % 2][h][:],
                                     in_=e1t[h * 128:(h + 1) * 128,
                                             col0:col0 + EMB_COLS]
                                     ).then_inc(s_ld[0][c % 2], 16)
            for pc in range(2):  # first idx pairs, before the drain stream
                c0 = pc * sc.piece_cols
                for side in range(2):
                    scalar.dma_start(out=idxt[side][:, c0:c0 + sc.piece_cols],
                                     in_=(idxa, idxb)[side][:, c0:c0 + sc.piece_cols]
                                     ).then_inc(s_ci[pc], 16)
            for q, (tab, g) in enumerate(GSEQ):
                if q < EARLY_DRAINS:
                    emit_pwrite(scalar, q)  # drain happens on the DVE
                    continue
                if q == EARLY_DRAINS:
                    scalar.wait_ge(s_dr, EARLY_DRAINS)  # order after DVE drains
                scalar.wait_ge(s_mm, mmc[q])
                if q >= 4:
                    scalar.wait_ge(s_pw[q % 4], 16 * (q // 4))
                scalar.copy(out=pv[q % 4][:], in_=ps[q % 4][:]).then_inc(s_dr, 1)
                emit_pwrite(scalar, q)

        # ------------------------------------------------ Pool: gathers
        @block.gpsimd
        def _(g_eng):
            from concourse import library_config
            g_eng.load_library(library_config.mlp)
            regs = {n: g_eng.to_reg(n) for n in
                    sorted({c for calls in sc.bcalls for c in calls})}

            g_eng.wait_ge(s_cl, BASE_CONSTS)
            gated = -1
            ci_gated = 0
            for bi, (bk, calls, off) in enumerate(sc.batches):
                fi, ti = bk // NT, bk % NT
                pos = BUCKET_POS[bk]
                if pos > gated:
                    for r, n in enumerate(pw_lane_counts(pos)):
                        if n:
                            g_eng.wait_ge(s_pw[r], 16 * n)
                    gated = pos
                need_pairs = sc.idx_gate[bk] // 32
                while ci_gated < need_pairs:
                    g_eng.wait_ge(s_ci[ci_gated], 32)
                    ci_gated += 1
                if bi >= 2:
                    g_eng.wait_ge(s_f1, bi - 1)  # at/btg[bi%2] consumed
                coff = 0
                for n in calls:
                    col0 = (sc.cap_off[bk] + off + coff) // 16
                    so = coff // 128
                    S = n // 128
                    for side, buf, tbl, pi in ((0, at, 0, fi), (1, btg, 1, ti)):
                        g_eng.dma_gather(
                            out_ap=buf[bi % 2][:, so * 128:(so + S) * 128]
                                .bitcast(I32).rearrange("p (s d) -> p s d", d=64),
                            in_ap=pg[tbl][poff[tbl][pi]:poff[tbl][pi + 1], :],
                            idxs_ap=idxt[side][:, col0:col0 + n // 16],
                            num_idxs=n, num_idxs_reg=regs[n], elem_size=64,
                            queue_num=0,
                        ).then_inc(s_g[bi % 2], 16)
                    coff += n

        # ------------------------------------------------ DVE: mul + folds + red
        @block.vector
        def _(vector):
            with nc.allow_low_precision(reason="fp16 dot products; 2e-2 tol"):
                def drain(q):
                    vector.wait_ge(s_mm, mmc[q])
                    if q >= 4:
                        vector.wait_ge(s_pw[q % 4], 16 * (q // 4))
                    vector.tensor_copy(out=pv[q % 4][:],
                                       in_=ps[q % 4][:]).then_inc(s_dr, 1)

                # early drains run before the batch loop; interleaving
                # batches into the chain was tried and stalls the projection
                # front (DRAIN_BEFORE_BATCH kept for experimentation)
                drained = 0
                ndv = 0
                for bi, (bk, calls, off) in enumerate(sc.batches):
                    target = DRAIN_BEFORE_BATCH.get(bi, EARLY_DRAINS if bi > 4
                                                    else EARLY_DRAINS)
                    while drained < target:
                        drain(drained)
                        drained += 1
                    sz = sum(calls)
                    vector.wait_ge(s_g[bi % 2], 16 * sc.gthresh[bi])
                    a2 = at[bi % 2][:, :sz]
                    vector.tensor_mul(out=a2, in0=a2,
                                      in1=btg[bi % 2][:, :sz]).then_inc(s_dv, 1)
                    ndv += 1
                    a3 = at[bi % 2][:, :sz].rearrange("p (s d) -> p s d", d=128)
                    vector.wait_ge(s_dv, ndv)
                    vector.tensor_add(
                        out=f1b[bi % 2][:, :sz // 2].rearrange("p (s d) -> p s d", d=64),
                        in0=a3[:, :, 0:64], in1=a3[:, :, 64:128],
                    ).then_inc(s_f1, 1)
                    f1v = f1b[bi % 2][:, :sz // 2].rearrange("p (s d) -> p s d", d=64)
                    vector.wait_ge(s_f1, bi + 1)
                    vector.tensor_add(
                        out=f2b[bi % 2][:, :sz // 4].rearrange("p (s d) -> p s d", d=32),
                        in0=f1v[:, :, 0:32], in1=f1v[:, :, 32:64],
                    ).then_inc(s_dv, 1)
                    ndv += 1
                    f2v = f2b[bi % 2][:, :sz // 4].rearrange("p (s d) -> p s d", d=32)
                    vector.wait_ge(s_dv, ndv)
                    vector.tensor_add(
                        out=f3b[bi % 2][:, :sz // 8].rearrange("p (s d) -> p s d", d=16),
                        in0=f2v[:, :, 0:16], in1=f2v[:, :, 16:32],
                    ).then_inc(s_dv, 1)
                    ndv += 1
                    f3v = f3b[bi % 2][:, :sz // 8].rearrange("p (s d) -> p s d", d=16)
                    vector.wait_ge(s_dv, ndv)
                    vector.tensor_add(
                        out=f4b[bi % 2][:, :sz // 16].rearrange("p (s d) -> p s d", d=8),
                        in0=f3v[:, :, 0:8], in1=f3v[:, :, 8:16],
                    ).then_inc(s_dv, 1)
                    ndv += 1
                    vector.wait_ge(s_dv, ndv)
                    so = off // 128
                    vector.reduce_sum(
                        out=rt[bk][:, so:so + sz // 128],
                        in_=f4b[bi % 2][:, :sz // 16].rearrange("p (s d) -> p s d", d=8),
                        axis=AX.X,
                    ).then_inc(s_red, 1)

    return nc


_NC_CACHE = {}
_SCHED_CACHE = {}


def _get_sched(caps):
    key = tuple(caps)
    if key not in _SCHED_CACHE:
        _SCHED_CACHE[key] = make_sched(list(caps))
    return _SCHED_CACHE[key]


def _get_nc(has_bias, sc):
    key = (has_bias, tuple(sc.caps))
    if key not in _NC_CACHE:
        nc = build_bass(has_bias, sc)
        from concourse.library_overlay import lower_extended_insts
        lower_extended_insts(nc)
        _NC_CACHE[key] = nc
    return _NC_CACHE[key]


# ---------------------------------------------------------------- host side
def _to_f16(x):
    return np.ascontiguousarray(np.asarray(x, dtype=np.float32)).astype(np.float16)


def _marshal(emb_1, emb_2, nodes_from_to, W1, b1, W2, b2):
    """Shard/bucket inputs per core.  Returns (in_maps, books, has_bias, sc)."""
    f = np.asarray(nodes_from_to[:, 0], dtype=np.int64)
    t = np.asarray(nodes_from_to[:, 1], dtype=np.int64)
    emb_1 = np.asarray(emb_1, dtype=np.float32)
    emb_2 = np.asarray(emb_2, dtype=np.float32)
    W1h = _to_f16(W1)
    W2h = _to_f16(W2)
    b1v = np.asarray(b1, dtype=np.float32).reshape(-1)
    b2v = np.asarray(b2, dtype=np.float32).reshape(-1)
    has_bias = bool(b1v.any() or b2v.any())
    b1h = b1v.astype(np.float16).reshape(1, D_OUT)
    b2h = b2v.astype(np.float16).reshape(1, D_OUT)
    onesh = np.ones((1, 128), np.float16)

    core = (f // FSPAN) * 4 + t // TSPAN
    order0 = np.argsort(core, kind="stable")
    ccnt = np.bincount(core, minlength=N_CORES)
    coff = np.concatenate([[0], np.cumsum(ccnt)])

    # column-permuted transposed embedding shards:
    #   e_t[:, tile*128 + p] = emb_local[(tile//4)*512 + 4*p + (tile%4)]
    def build_embt(emb, lo, span, rows):
        ntile = rows // 128
        tiles = np.arange(ntile)
        cols_row = ((tiles[:, None] // 4) * 512 + 4 * np.arange(128)[None, :]
                    + (tiles[:, None] % 4)).reshape(-1)  # table row per column
        out = np.zeros((D_IN, rows), np.float16)
        valid = cols_row < span
        src = emb[lo:lo + span]
        out[:, valid] = src[cols_row[valid]].T.astype(np.float16)
        return out

    f_off = np.asarray(F_OFF)
    t_off = np.asarray(T_OFF)

    # bucket counts per core to pick the capacity tier
    percore = []
    for c in range(N_CORES):
        a, b = c // 4, c % 4
        sel = order0[coff[c]:coff[c + 1]]
        fl = f[sel] - a * FSPAN
        tl = t[sel] - b * TSPAN
        fi = np.searchsorted(f_off, fl, side="right") - 1
        ti = np.searchsorted(t_off, tl, side="right") - 1
        bk = fi * NT + ti
        o2 = np.argsort(bk, kind="stable")
        percore.append((sel[o2], bk[o2],
                        np.bincount(bk, minlength=NBUCKET)))

    maxcnt = np.max([p[2] for p in percore], axis=0)
    caps = [int(math.ceil(max(128, c) / 128) * 128) for c in maxcnt]
    sc = _get_sched(caps)

    in_maps, books = [], []
    for c in range(N_CORES):
        a, b = c // 4, c % 4
        sel2, bk2, cnts = percore[c]
        fl2 = f[sel2] - a * FSPAN
        tl2 = t[sel2] - b * TSPAN
        pos = np.concatenate([[0], np.cumsum(cnts)])

        ia = np.zeros(sc.pad_edges, np.int16)
        ib = np.zeros(sc.pad_edges, np.int16)
        for k in range(NBUCKET):
            i_, j_ = k // NT, k % NT
            n = cnts[k]
            o = sc.cap_off[k]
            ia[o:o + n] = (fl2[pos[k]:pos[k + 1]] - F_OFF[i_]).astype(np.int16)
            ib[o:o + n] = (tl2[pos[k]:pos[k + 1]] - T_OFF[j_]).astype(np.int16)
        # wrap by 16 per gather call; replicate content across partition groups
        wrapped_a = np.zeros((128, sc.idx_cols), np.int16)
        wrapped_b = np.zeros((128, sc.idx_cols), np.int16)
        for k in range(NBUCKET):
            o = sc.cap_off[k]
            coffs = 0
            for n in sc.bcalls[k]:
                seg_a = ia[o + coffs:o + coffs + n].reshape(n // 16, 16).T
                seg_b = ib[o + coffs:o + coffs + n].reshape(n // 16, 16).T
                c0 = (o + coffs) // 16
                wrapped_a[:, c0:c0 + n // 16] = np.tile(seg_a, (8, 1))
                wrapped_b[:, c0:c0 + n // 16] = np.tile(seg_b, (8, 1))
                coffs += n

        in_maps.append({
            "e1t": build_embt(emb_1, a * FSPAN, FSPAN, P1_ROWS),
            "e2t": build_embt(emb_2, b * TSPAN, TSPAN, P2_ROWS),
            "w1": W1h, "w2": W2h, "b1c": b1h, "b2c": b2h, "onesd": onesh,
            "idxa": wrapped_a, "idxb": wrapped_b,
        })
        books.append((sel2, cnts, pos))
    return in_maps, books, has_bias, sc


def _unmarshal(results, books, n_edges, sc):
    out = np.empty(n_edges, np.float32)
    for c in range(N_CORES):
        sel2, cnts, pos = books[c]
        r = results[c]["res"]  # [128, res_cols] f16
        vals = np.asarray(r).astype(np.float32).T.reshape(-1)  # e = s*128+p
        for k in range(NBUCKET):
            n = cnts[k]
            if n == 0:
                continue
            o = sc.cap_off[k]
            out[sel2[pos[k]:pos[k + 1]]] = vals[o:o + n]
    return out


def _run(inputs, trace=False, **run_kwargs):
    from concourse.bass_utils import run_bass_kernel_spmd

    in_maps, books, has_bias, sc = _marshal(**inputs)
    nc = _get_nc(has_bias, sc)
    r = run_bass_kernel_spmd(
        nc, in_maps, core_ids=list(range(N_CORES)), trace=trace, **run_kwargs
    )
    out = _unmarshal(r.results, books, len(inputs["nodes_from_to"]), sc)
    return out, r


def kernel(**inputs) -> np.ndarray:
    out, _ = _run(inputs, trace=False)
    return out
